# revision 36
# baseline (speedup 1.0000x reference)
"""Trainium2 Bass kernel for nn_ArcDecoderLayer (sparse_attention).

Self-contained: takes FULL unsharded inputs, shards across 8 NeuronCores
(head-parallel attention, row-parallel o_proj with AllReduce, FF-parallel
MLP with AllGather of the intermediate), returns the FULL output.

v2 layout/schedule:
- LN1/LN2 statistics as concurrent col-tiled (sum, sumsq) matmul pairs;
  raw sums broadcast with 1/D folded into the broadcast constant so all
  stats math runs partition-aligned on full tiles.
- Attention: per-key-tile waves; score MMs row-tiled concurrent pairs,
  AV MMs col-tiled concurrent pairs, denominator MMs concurrent pairs,
  software-pipelined one key-tile ahead of the ScalarE exp; diagonal
  band tiles narrowed to their unmasked column range; packed full-width
  RoPE.
- o_proj computed locally from this core's heads into a full-D partial;
  per-block ReduceScatter(+AllGather) gives the residual rows and the
  full h sum (no attn/o AllGathers).
- MLP: gate/up per 512-block; row-parallel down-proj straight from the
  SBUF-resident m slice into a full-D partial, ReduceScattered back (no
  m AllGather, no 33MB of gathered-m reads).
All matmul compute bf16 with f32 PSUM accumulation.
"""

import sys
import types

sys.path.insert(0, "/opt/trn_rl_repo")

# ---- shim antenv.axon_hooks so trace=True profiling works in this image ----
if "antenv.axon_hooks" not in sys.modules:
    _hook_mod = types.ModuleType("antenv.axon_hooks")
    _hook_state = {"hook": None}

    def _set_hook(h):
        _hook_state["hook"] = h

    def _get_hook():
        return _hook_state["hook"]

    _hook_mod.set_axon_ntff_profile_hook = _set_hook
    _hook_mod.get_axon_ntff_profile_hook = _get_hook
    sys.modules["antenv.axon_hooks"] = _hook_mod
    try:
        import antenv

        antenv.axon_hooks = _hook_mod
        from trn_agent_boot.trn_boot import _ntff_profile_via_ctypes

        _set_hook(_ntff_profile_via_ctypes("/opt/axon/libaxon_pjrt.so"))
    except Exception:
        pass

import numpy as np
import ml_dtypes

import concourse.bass as bass
import concourse.mybir as mybir
import concourse.tile as tile
from concourse import library_config
from concourse.vector_clock import ScopedClock

BF16 = ml_dtypes.bfloat16

N_CORES = 8
D = 2048
FF = 8192
H = 32
DH = 64
RD = 16
EPS = 1e-5
BASE = 10000.0

J = D // N_CORES        # 256 head-dims per core (4 heads)
FFL = FF // N_CORES     # 1024 ff dims per core
KC = D // 128           # 16 contraction chunks
NBLK = 512              # lq block width
MD = J // 128           # 2 output Mtiles per core for down/out


WAIT_LIMITS = {"InstNoOp": 1, "InstDrain": 1, "InstEventSemaphore": 1}
DEFAULT_WAIT_LIMIT = 1


class PatchedTC(tile.TileContext):
    """TileContext patched for this walrus build, which rejects instructions
    carrying more than a couple of sync wait commands: excess waits are
    split onto injected same-engine nops just before the instruction."""

    _wsplit_n = 0

    def _split_excess_waits(self, ordered):
        for bb, insts in ordered.items():
            out = []
            for inst in insts:
                si = inst.sync_info
                waits = list(si.on_wait) if si and si.on_wait else []
                lim = WAIT_LIMITS.get(type(inst).__name__,
                                      DEFAULT_WAIT_LIMIT)
                if len(waits) > lim:
                    for w in waits[:-lim]:
                        nop = mybir.InstNoOp(
                            name=f"I-wsplit-{PatchedTC._wsplit_n}",
                            ins=[], outs=[], engine=inst.engine,
                            nofuse=True)
                        PatchedTC._wsplit_n += 1
                        nop.sync_info = mybir.SyncInfo(
                            on_wait=[w], on_update=[])
                        out.append(nop)
                    inst.sync_info = mybir.SyncInfo(
                        on_wait=waits[-lim:],
                        on_update=list(si.on_update or []))
                out.append(inst)
            ordered[bb] = out

    def _lower_ordered_insts(self, ordered):
        self._split_excess_waits(ordered)
        return super()._lower_ordered_insts(ordered)

    def _drain_and_barrier(self, tick_clock, wait_clock):
        nc = self.nc
        probe = nc.sync.nop(nofuse=True, hint="tail_wait_probe")
        wait_clock.add_sem_waits(
            probe.ins, ScopedClock({None: tick_clock.global_clock})
        )
        waits = list(probe.ins.sync_info.on_wait or [])
        probe.ins.sync_info.on_wait = waits[:1]
        for i in range(1, len(waits)):
            n = nc.sync.nop(nofuse=True, hint=f"tail_wait_{i}")
            n.ins.sync_info = mybir.SyncInfo(on_wait=[waits[i]], on_update=[])
        nc.sync.drain()
        nc.all_engine_barrier()
        assert self.sems is not None
        popped = nc._tile_sem_poison_stack.pop()
        assert popped is self._sem_poison
        nc.clear_and_free_semaphores(list(self.sems.allocated().values()))
        nc.all_engine_barrier()


def build_graph(S):
    """Build the SPMD 8-core graph for sequence length S (multiple of 512)."""
    dt = mybir.dt
    f32, bf16 = dt.float32, dt.bfloat16
    AF = mybir.ActivationFunctionType
    Alu = mybir.AluOpType
    NB = S // NBLK          # lq blocks (4)
    LT = S // 128           # 128-wide l tiles per part
    S2 = 2 * S

    nc = bass.Bass()
    P = nc.declare_dram_parameter

    xm_e = P("xm", [128, KC, S], bf16, isOutput=False)
    xh_e = P("xh", [128, KC, S], bf16, isOutput=False)
    ident8_e = P("ident8", [128, 128], bf16, isOutput=False)
    wq_e = P("wq", [128, KC, J], bf16, isOutput=False)
    wk_e = P("wk", [128, KC, J], bf16, isOutput=False)
    wv_e = P("wv", [128, KC, J], bf16, isOutput=False)
    wo_e = P("wo_p", [128, MD, D], bf16, isOutput=False)
    wg_e = P("wg", [128, KC, FFL], bf16, isOutput=False)
    wu_e = P("wu", [128, KC, FFL], bf16, isOutput=False)
    wd_e = P("wd", [128, FFL // 128, D], bf16, isOutput=False)
    # column (per-partition) weight rowsums + biases for q/k/vTh epilogues
    wsq_e = P("wsq", [128, 2], f32, isOutput=False)
    wsk_e = P("wsk", [128, 2], f32, isOutput=False)
    wsvc_e = P("wsvc", [128, 2], f32, isOutput=False)   # for vT_h epilogue
    bq_e = P("bq", [128, 2], f32, isOutput=False)
    bk_e = P("bk", [128, 2], f32, isOutput=False)
    bvc_e = P("bvc", [128, 2], f32, isOutput=False)
    # row layouts for v_mem epilogue
    wsv_e = P("wsv_row", [1, J], f32, isOutput=False)
    bv_e = P("bv_row", [1, J], f32, isOutput=False)
    bg_e = P("bg", [128, FFL // 128], f32, isOutput=False)
    bu_e = P("bu", [128, FFL // 128], f32, isOutput=False)
    wsg_e = P("wsg", [128, FFL // 128], f32, isOutput=False)
    wsu_e = P("wsu", [128, FFL // 128], f32, isOutput=False)
    ropec_e = P("rope_cos", [128, S2], bf16, isOutput=False)
    ropes_e = P("rope_sinsg", [128, S2], bf16, isOutput=False)
    masks_e = P("masks", [128, 4, NBLK], bf16, isOutput=False)
    out_e = P("out", [MD, 128, S], f32, isOutput=True)

    rg = [list(range(N_CORES))]

    with PatchedTC(nc) as tc:
        with (
            tc.tile_pool(name="const", bufs=1) as constp,
            tc.tile_pool(name="dram", bufs=1, space="DRAM") as dramp,
            tc.tile_pool(name="dsh", bufs=1, space="DRAM") as dshp,
        ):
            # first half of the gate weight lives below kqvp on the
            # pool stack so it can prefetch during attention and survive
            # into the MLP loop (releases stay LIFO)
            wguh1p = tc.alloc_tile_pool(name="wguh1", bufs=1)
            wg_h1 = wguh1p.tile([128, KC, FFL // 2], bf16)
            # block-0 h + LN2 stats, precomputed inside attention so the
            # MLP can start the moment attention drains
            h0p = tc.alloc_tile_pool(name="h0", bufs=1)
            h0_t = h0p.tile([128, KC, NBLK], bf16)
            rstd05 = h0p.tile([128, NBLK], bf16)
            c05 = h0p.tile([128, NBLK], bf16)
            kqvp = tc.alloc_tile_pool(name="kqv", bufs=1)
            ones_c = constp.tile([128, 1], bf16)
            nc.vector.memset(ones_c[:], 1.0)
            ones128 = constp.tile([128, 128], bf16)
            nc.vector.memset(ones128[:], 1.0)
            invD128 = constp.tile([128, 128], bf16)
            nc.vector.memset(invD128[:], 1.0 / D)
            eps_c = constp.tile([128, 1], f32)
            nc.vector.memset(eps_c[:], EPS)
            onesf = constp.tile([1, 128], f32)
            nc.vector.memset(onesf[:], 1.0)
            ident8 = constp.tile([128, 128], bf16)
            nc.gpsimd.dma_start(ident8[:], ident8_e[:])

            def bcast_rows(dst, src_row, width, pspool, ones_row):
                """dst[0:128, :width] = src_row[0, :width] via K=1 matmuls
                (partition_broadcast is not encodable by this walrus)."""
                for i in range(0, width, NBLK):
                    w = min(NBLK, width - i)
                    ps = pspool.tile([128, NBLK], f32, name="bc_ps",
                                     tag="bc_ps", bufs=1)
                    nc.tensor.matmul(ps[:, :w], ones_row[0:1, :],
                                     src_row[0:1, i:i + w],
                                     start=True, stop=True)
                    nc.vector.tensor_copy(dst[:, i:i + w], ps[:, :w])

            wsvb = constp.tile([128, J], f32)
            wsv_row = constp.tile([1, J], f32)
            nc.sync.dma_start(wsv_row[:], wsv_e[:])
            bvb = constp.tile([128, J], f32)
            bv_row = constp.tile([1, J], f32)
            nc.sync.dma_start(bv_row[:], bv_e[:])
            wsq_t = constp.tile([128, 2], f32)
            nc.sync.dma_start(wsq_t[:], wsq_e[:])
            wsk_t = constp.tile([128, 2], f32)
            nc.sync.dma_start(wsk_t[:], wsk_e[:])
            wsvc_t = constp.tile([128, 2], f32)
            nc.sync.dma_start(wsvc_t[:], wsvc_e[:])
            bq_t = constp.tile([128, 2], f32)
            nc.sync.dma_start(bq_t[:], bq_e[:])
            bk_t = constp.tile([128, 2], f32)
            nc.sync.dma_start(bk_t[:], bk_e[:])
            bvc_t = constp.tile([128, 2], f32)
            nc.sync.dma_start(bvc_t[:], bvc_e[:])
            bg_t = constp.tile([128, FFL // 128], f32)
            nc.sync.dma_start(bg_t[:], bg_e[:])
            bu_t = constp.tile([128, FFL // 128], f32)
            nc.sync.dma_start(bu_t[:], bu_e[:])
            wsg_t = constp.tile([128, FFL // 128], f32)
            nc.sync.dma_start(wsg_t[:], wsg_e[:])
            wsu_t = constp.tile([128, FFL // 128], f32)
            nc.sync.dma_start(wsu_t[:], wsu_e[:])

            # persistent QKV outputs
            kT = [kqvp.tile([128, S2], bf16, name=f"kT{m}") for m in range(2)]
            qT = [kqvp.tile([128, S], bf16, name=f"qT{m}") for m in range(2)]
            vTh = [kqvp.tile([128, S], bf16, name=f"vTh{m}") for m in range(2)]
            v_mem = kqvp.tile([128, LT, J], bf16)

            # v_mem epilogue needs column-layout stats of the mem part
            rstd_col_mem = constp.tile([128, LT], f32)
            c_col_mem = constp.tile([128, LT], f32)

            # rope tables (persistent through phase 1)
            rope_loaded = [False]

            def load_rope_tables():
                if not rope_loaded[0]:
                    nc.sync.dma_start(cos_t[:], ropec_e[:, 0:S])
                    nc.sync.dma_start(sin_t[:], ropes_e[:, 0:S])
                    rope_loaded[0] = True

            def rope_packed(slices, name):
                """Apply partial rotary to up to 8 (tile, col_lo) 16-row
                rotary groups at once, packed across all 128 partitions.
                Each slice is (tile, row_lo, col_lo); processes
                tile[row_lo:row_lo+16, col_lo:col_lo+S]. The cos/sin
                tables have the same 16-row pattern on every group, and
                positions repeat across both S-halves."""
                pk = ropep.tile([128, S], bf16, name=f"pk_{name}",
                                tag="rope_pk", bufs=1)
                sw = ropep.tile([128, S], bf16, name=f"sw_{name}",
                                tag="rope_sw", bufs=1)
                for i, (t, rlo, clo) in enumerate(slices):
                    csl = slice(clo, clo + S)
                    eng = nc.sync if i % 2 == 0 else nc.scalar
                    eng.dma_start(pk[16 * i:16 * i + 16, :],
                                  t[rlo:rlo + 16, csl])
                    eng.dma_start(sw[16 * i:16 * i + 8, :],
                                  t[rlo + 8:rlo + 16, csl])
                    eng.dma_start(sw[16 * i + 8:16 * i + 16, :],
                                  t[rlo:rlo + 8, csl])
                n = 16 * len(slices)
                nc.vector.tensor_mul(pk[:n, :], pk[:n, :], cos_t[:n, :])
                nc.vector.tensor_mul(sw[:n, :], sw[:n, :], sin_t[:n, :])
                nc.vector.tensor_add(pk[:n, :], pk[:n, :], sw[:n, :])
                for i, (t, rlo, clo) in enumerate(slices):
                    csl = slice(clo, clo + S)
                    eng = nc.sync if i % 2 == 0 else nc.scalar
                    eng.dma_start(t[rlo:rlo + 16, csl],
                                  pk[16 * i:16 * i + 16, :])

            # ---------- LN1 stats (row-major, col-tiled pairs) -------------
            def stats_nb(x_nb, nb, sqp, psp, row16):
                """Per 512-block LN sums: sum -> stp[0:1] ((0,0)),
                sumsq -> stp[32:33] ((0,32)): concurrent col-tiled pairs
                at different partitions of one PSUM bank."""
                nsl = slice(nb * NBLK, (nb + 1) * NBLK)
                stp = psp.tile([128, NBLK], f32, name="stp")
                for kc in range(KC):
                    sq_t = sqp.tile([128, NBLK], bf16, name="sq_t")
                    nc.scalar.activation(sq_t[:], x_nb[:, kc, :],
                                         AF.Square)
                    nc.tensor.matmul(
                        stp[0:1, :], ones_c[:], x_nb[:, kc, :],
                        start=(kc == 0), stop=(kc == KC - 1),
                        tile_position=(0, 0))
                    nc.tensor.matmul(
                        stp[32:33, :], ones_c[:], sq_t[:],
                        start=(kc == 0), stop=(kc == KC - 1),
                        tile_position=(0, 32))
                nc.vector.tensor_copy(row16[0:1, nsl], stp[0:1, :])
                nc.vector.tensor_copy(row16[32:33, nsl], stp[32:33, :])

            def ln_stats_fin(row16, psp, rowp, part_name, want_col):
                """Broadcast the raw sums (1/D folded into the broadcast
                constant) and finish the stats math on full [128, S]
                tiles. Returns (rstd_b, c_b)."""
                mean_b = rowp.tile([128, S], bf16, name="mean_b",
                                   tag="meanb")
                ex2_b = rowp.tile([128, S], bf16, name="ex2_b", tag="ex2b")
                for i in range(0, S, NBLK):
                    isl = slice(i, i + NBLK)
                    ps = psp.tile([128, NBLK], f32, name="bc_ps",
                                  tag="bc_ps", bufs=1)
                    nc.tensor.matmul(ps[:], invD128[0:1, :],
                                     row16[0:1, isl],
                                     start=True, stop=True)
                    nc.vector.tensor_copy(mean_b[:, isl], ps[:])
                    ps2 = psp.tile([128, NBLK], f32, name="bc_ps2",
                                   tag="bc_ps", bufs=1)
                    nc.tensor.matmul(ps2[:], invD128[32:33, :],
                                     row16[32:33, isl],
                                     start=True, stop=True)
                    nc.vector.tensor_copy(ex2_b[:, isl], ps2[:])
                m2_b = rowp.tile([128, S], bf16, name="m2_b", tag="m2b")
                nc.vector.tensor_mul(m2_b[:], mean_b[:], mean_b[:])
                nc.vector.tensor_sub(ex2_b[:], ex2_b[:], m2_b[:])
                rstd_b = rowp.tile([128, S], bf16, name="rstd_b",
                                   tag="rstdb")
                nc.scalar.activation(ex2_b[:], ex2_b[:], AF.Ln,
                                     bias=eps_c[:])
                nc.scalar.activation(rstd_b[:], ex2_b[:], AF.Exp,
                                     scale=-0.5)
                c_b = rowp.tile([128, S], bf16, name="c_b", tag="cb")
                nc.vector.tensor_mul(c_b[:], mean_b[:], rstd_b[:])
                if want_col:
                    # round-trip on the vector queue so the xh loads on
                    # sync/scalar are not stuck behind this stats chain
                    for nm, row, col in (("rstd", rstd_b, rstd_col_mem),
                                         ("c", c_b, c_col_mem)):
                        dr = dramp.tile([S], bf16,
                                        name=f"st_{nm}_{part_name}")
                        nc.gpsimd.dma_start(
                            dr[:].rearrange("(o a) -> o a", o=1),
                            row[0:1, :])
                        col16 = rowp.tile([128, LT], bf16,
                                          name=f"c16_{nm}", tag="col16")
                        nc.gpsimd.dma_start(
                            col16[:],
                            dr[:].rearrange("(t p) -> p t", p=128))
                        nc.vector.tensor_copy(col[:], col16[:])
                return rstd_b, c_b

            def fin_nb(row16, nb, scrp, psp):
                """Per-512-block stats finalize: broadcast raw sums and
                produce (rstd, c) tiles for just these columns, so the
                epilogues + rope for block nb can run while later blocks
                still project."""
                nsl = slice(nb * NBLK, (nb + 1) * NBLK)
                mean5 = scrp.tile([128, NBLK], bf16, name="mean5",
                                  tag="f_mean", bufs=2)
                ex25 = scrp.tile([128, NBLK], bf16, name="ex25",
                                 tag="f_ex2", bufs=2)
                ps = psp.tile([128, NBLK], f32, name="bc_ps",
                              tag="bc_ps", bufs=1)
                nc.tensor.matmul(ps[:], invD128[0:1, :], row16[0:1, nsl],
                                 start=True, stop=True)
                nc.vector.tensor_copy(mean5[:], ps[:])
                ps2 = psp.tile([128, NBLK], f32, name="bc_ps2",
                               tag="bc_ps", bufs=1)
                nc.tensor.matmul(ps2[:], invD128[32:33, :],
                                 row16[32:33, nsl],
                                 start=True, stop=True)
                nc.vector.tensor_copy(ex25[:], ps2[:])
                m25 = scrp.tile([128, NBLK], bf16, name="m25",
                                tag="f_m2", bufs=2)
                nc.vector.tensor_mul(m25[:], mean5[:], mean5[:])
                nc.vector.tensor_sub(ex25[:], ex25[:], m25[:])
                nc.scalar.activation(ex25[:], ex25[:], AF.Ln,
                                     bias=eps_c[:])
                rstd5 = scrp.tile([128, NBLK], bf16, name="rstd5",
                                  tag="f_rstd", bufs=2)
                nc.scalar.activation(rstd5[:], ex25[:], AF.Exp,
                                     scale=-0.5)
                c5 = scrp.tile([128, NBLK], bf16, name="c5",
                               tag="f_c", bufs=2)
                nc.vector.tensor_mul(c5[:], mean5[:], rstd5[:])
                return rstd5, c5

            def proj_epi_nb(dst, dst_off, nb, rstd5, c5, ws_t, b_t,
                            scrp):
                """LN epilogue for one 512-block: d = d*rstd-(c*ws-b)."""
                for m in range(2):
                    d = dst[m][:, dst_off + nb * NBLK:
                               dst_off + (nb + 1) * NBLK]
                    cw = scrp.tile([128, NBLK], bf16, name="cw_nb",
                                   tag="cw_nb", bufs=2)
                    nc.vector.tensor_scalar(
                        out=cw[:], in0=c5[:],
                        scalar1=ws_t[:, m:m + 1],
                        scalar2=b_t[:, m:m + 1],
                        op0=Alu.mult, op1=Alu.subtract)
                    nc.vector.tensor_mul(d, d, rstd5[:])
                    nc.vector.tensor_sub(d, d, cw[:])

            def rope_packed_nb(slices, name, nb):
                """rope_packed restricted to one 512-column block."""
                nsl = slice(nb * NBLK, (nb + 1) * NBLK)
                pk = ropep.tile([128, NBLK], bf16, name=f"pk_{name}",
                                tag="rope_pk", bufs=2)
                sw = ropep.tile([128, NBLK], bf16, name=f"sw_{name}",
                                tag="rope_sw", bufs=2)
                for i, (t, rlo, clo) in enumerate(slices):
                    csl = slice(clo + nb * NBLK, clo + (nb + 1) * NBLK)
                    eng = nc.sync if i % 2 == 0 else nc.scalar
                    eng.dma_start(pk[16 * i:16 * i + 16, :],
                                  t[rlo:rlo + 16, csl])
                    eng.dma_start(sw[16 * i:16 * i + 8, :],
                                  t[rlo + 8:rlo + 16, csl])
                    eng.dma_start(sw[16 * i + 8:16 * i + 16, :],
                                  t[rlo:rlo + 8, csl])
                n = 16 * len(slices)
                nc.vector.tensor_mul(pk[:n, :], pk[:n, :],
                                     cos_t[:n, nsl])
                nc.vector.tensor_mul(sw[:n, :], sw[:n, :],
                                     sin_t[:n, nsl])
                nc.vector.tensor_add(pk[:n, :], pk[:n, :], sw[:n, :])
                for i, (t, rlo, clo) in enumerate(slices):
                    csl = slice(clo + nb * NBLK, clo + (nb + 1) * NBLK)
                    eng = nc.sync if i % 2 == 0 else nc.scalar
                    eng.dma_start(t[rlo:rlo + 16, csl],
                                  pk[16 * i:16 * i + 16, :])

            def proj_raw_nb(wt, dst, dst_off, x_nb, nb, psp):
                """Raw projection matmuls for one 512-column block,
                copied to dst bf16 (no LN dependency)."""
                for m in range(2):
                    ps = psp.tile([128, NBLK], f32, name="proj_ps")
                    for kc in range(KC):
                        nc.tensor.matmul(
                            ps[:],
                            wt[:, kc, m * 128:(m + 1) * 128],
                            x_nb[:, kc, :],
                            start=(kc == 0), stop=(kc == KC - 1))
                    d = dst[m][:, dst_off + nb * NBLK:
                               dst_off + (nb + 1) * NBLK]
                    nc.vector.tensor_copy(d, ps[:])

            def proj_epi(dst, dst_off, rstd_b, c_b, ws_t, b_t, scrp):
                """LN epilogue in place: d = d*rstd - (c*ws - bias)."""
                for m in range(2):
                    for nb in range(NB):
                        sl = slice(nb * NBLK, (nb + 1) * NBLK)
                        d = dst[m][:, dst_off + nb * NBLK:
                                   dst_off + (nb + 1) * NBLK]
                        cw = scrp.tile([128, NBLK], bf16, name="cw_nb",
                                       tag="cw_nb", bufs=2)
                        nc.vector.tensor_scalar(
                            out=cw[:], in0=c_b[:, sl],
                            scalar1=ws_t[:, m:m + 1],
                            scalar2=b_t[:, m:m + 1],
                            op0=Alu.mult, op1=Alu.subtract)
                        nc.vector.tensor_mul(d, d, rstd_b[:, sl])
                        nc.vector.tensor_sub(d, d, cw[:])

            with (
                tc.tile_pool(name="wqkv", bufs=1) as wqkvp,
                tc.tile_pool(name="psq", bufs=2, space="PSUM") as psqp,
                tc.tile_pool(name="psst", bufs=2, space="PSUM") as psstp,
            ):
                wq_t = wqkvp.tile([128, KC, J], bf16)
                wk_t = wqkvp.tile([128, KC, J], bf16)
                wv_t = wqkvp.tile([128, KC, J], bf16)

                # ----- phase 1: both parts with per-512-block x tiles.
                # The mem-part finalize (stats math + k/v epilogues, a
                # long serial DVE chain) is emitted inside the hid loop
                # so it overlaps the hid projection streams. -----
                ropep = tc.alloc_tile_pool(name="rope", bufs=1)
                cos_t = ropep.tile([128, S], bf16)
                sin_t = ropep.tile([128, S], bf16)
                with (
                    tc.tile_pool(name="xm", bufs=2) as xmp,
                    tc.tile_pool(name="sqa", bufs=8) as sqap,
                    tc.tile_pool(name="rowa", bufs=1) as rowap,
                ):
                    nc.sync.dma_start(wk_t[:], wk_e[:])
                    nc.scalar.dma_start(wv_t[:], wv_e[:])
                    nc.sync.dma_start(wq_t[:], wq_e[:])
                    row16a = rowap.tile([128, S], bf16, name="r16_mem",
                                        tag="r16m")
                    row16b = rowap.tile([128, S], bf16, name="r16_hid",
                                        tag="r16h")
                    for nb in range(NB):
                        x_nb = xmp.tile([128, KC, NBLK], bf16,
                                        name="x_nb")
                        for kc in range(KC):
                            eng = nc.sync if kc % 2 == 0 else nc.scalar
                            eng.dma_start(
                                x_nb[:, kc, :],
                                xm_e[:, kc, nb * NBLK:(nb + 1) * NBLK])
                        proj_raw_nb(wk_t, kT, 0, x_nb, nb, psqp)
                        # v_mem row-major: lhsT = xm l-tile, rhs = wv
                        for li in range(4):
                            lt = nb * 4 + li
                            ps = psqp.tile([128, J], f32, name="vm_ps",
                                           bufs=2)
                            for kc in range(KC):
                                nc.tensor.matmul(
                                    ps[:],
                                    x_nb[:, kc, li * 128:(li + 1) * 128],
                                    wv_t[:, kc, :],
                                    start=(kc == 0), stop=(kc == KC - 1))
                            nc.vector.tensor_copy(v_mem[:, lt, :], ps[:])
                        stats_nb(x_nb, nb, sqap, psstp, row16a)
                    for nb in range(NB):
                        x_nb = xmp.tile([128, KC, NBLK], bf16,
                                        name="x_nb")
                        for kc in range(KC):
                            eng = nc.sync if kc % 2 == 0 else nc.scalar
                            eng.dma_start(
                                x_nb[:, kc, :],
                                xh_e[:, kc, nb * NBLK:(nb + 1) * NBLK])
                        if nb == 0:
                            load_rope_tables()
                        proj_raw_nb(wq_t, qT, 0, x_nb, nb, psqp)
                        proj_raw_nb(wk_t, kT, S, x_nb, nb, psqp)
                        proj_raw_nb(wv_t, vTh, 0, x_nb, nb, psqp)
                        stats_nb(x_nb, nb, sqap, psstp, row16b)
                        # per-block hid finalize: epilogues + rope for
                        # block nb run while later blocks still project
                        rstd5, c5 = fin_nb(row16b, nb, sqap, psstp)
                        proj_epi_nb(qT, 0, nb, rstd5, c5, wsq_t, bq_t,
                                    sqap)
                        rope_packed_nb(
                            [(qT[0], 0, 0), (qT[0], 64, 0),
                             (qT[1], 0, 0), (qT[1], 64, 0),
                             (kT[0], 0, 0), (kT[0], 64, 0),
                             (kT[1], 0, 0), (kT[1], 64, 0)], "a", nb)
                        proj_epi_nb(kT, S, nb, rstd5, c5, wsk_t, bk_t,
                                    sqap)
                        rope_packed_nb(
                            [(kT[0], 0, S), (kT[0], 64, S),
                             (kT[1], 0, S), (kT[1], 64, S)], "b", nb)
                        proj_epi_nb(vTh, 0, nb, rstd5, c5, wsvc_t, bvc_t,
                                    sqap)
                        if nb == 0:
                            bcast_rows(wsvb, wsv_row, J, psqp, onesf)
                            bcast_rows(bvb, bv_row, J, psqp, onesf)
                        # per-block MEM finalize: k epilogue + v_mem
                        # epilogue for mem-block nb
                        rstd5m, c5m = fin_nb(row16a, nb, sqap, psstp)
                        proj_epi_nb(kT, 0, nb, rstd5m, c5m, wsk_t, bk_t,
                                    sqap)
                        # column-layout stats for the v_mem epilogue via
                        # a tiny DRAM round-trip on the gpsimd queue
                        for nm, row, col in (
                                ("rstd", rstd5m, rstd_col_mem),
                                ("c", c5m, c_col_mem)):
                            dr = dramp.tile([NBLK], bf16,
                                            name=f"st_{nm}_{nb}")
                            nc.gpsimd.dma_start(
                                dr[:].rearrange("(o a) -> o a", o=1),
                                row[0:1, :])
                            col4 = sqap.tile([128, 4], bf16,
                                             name=f"c4_{nm}",
                                             tag="col4", bufs=4)
                            nc.gpsimd.dma_start(
                                col4[:],
                                dr[:].rearrange("(t p) -> p t", p=128))
                            nc.vector.tensor_copy(
                                col[:, 4 * nb:4 * nb + 4], col4[:])
                        for li in range(4):
                            lt = nb * 4 + li
                            cwv = sqap.tile([128, J], f32,
                                            name="cwv", bufs=2)
                            nc.vector.tensor_scalar(
                                out=cwv[:], in0=wsvb[:],
                                scalar1=c_col_mem[:, lt:lt + 1],
                                scalar2=None, op0=Alu.mult)
                            nc.vector.scalar_tensor_tensor(
                                out=v_mem[:, lt, :],
                                in0=v_mem[:, lt, :],
                                scalar=rstd_col_mem[:, lt:lt + 1],
                                in1=cwv[:], op0=Alu.mult,
                                op1=Alu.subtract)
                            nc.vector.tensor_add(
                                v_mem[:, lt, :], v_mem[:, lt, :],
                                bvb[:])
                ropep.release()

            # ---------- loop 1: attention + local o_p + AllReduce ---------
            op_dram = [dramp.tile([D, NBLK], bf16, name=f"op_d{b}")
                       for b in range(NB)]
            h_sh = [dshp.tile([D, NBLK], bf16, name=f"h_sh{b}",
                              addr_space="Shared")
                    for b in range(NB)]
            with (
                tc.tile_pool(name="maskp", bufs=1) as maskp,
                tc.tile_pool(name="wop", bufs=1) as wop,
                tc.tile_pool(name="attw", bufs=8) as attwp,
                tc.tile_pool(name="attt", bufs=4) as atttp,
                tc.tile_pool(name="cmbp", bufs=3) as cmbp,
                tc.tile_pool(name="attr", bufs=1) as attrp,
                tc.tile_pool(name="oc", bufs=2) as ocp,
                tc.tile_pool(name="psS", bufs=3, space="PSUM") as psSp,
                tc.tile_pool(name="psA", bufs=1, space="PSUM") as psAp,
                tc.tile_pool(name="psD", bufs=1, space="PSUM") as psDp,
                tc.tile_pool(name="psH0", bufs=1,
                             space="PSUM") as psH0p,
            ):
                masks_t = maskp.tile([128, 4, NBLK], bf16)
                nc.sync.dma_start(masks_t[:], masks_e[:])
                h0sq_t = maskp.tile([128, KC, NBLK], bf16)
                wo_t = wop.tile([128, MD, D], bf16)
                nc.sync.dma_start(wo_t[:], wo_e[:])
                for b in range(NB):
                    bsl = slice(b * NBLK, (b + 1) * NBLK)
                    T = 4 * b + 4
                    if b == 1:
                        # prefetch the gate first half behind o stores
                        nc.sync.dma_start(wg_h1[:],
                                          wg_e[:, :, :FFL // 2])
                    if b == 3:
                        # h0 = o_sum(0) + xh(0) and its squares, built on
                        # DMA + gpsimd only (Scalar is block 3's pacer)
                        for tq in range(4):
                            eng = nc.sync if tq % 2 == 0 else nc.scalar
                            eng.dma_start(
                                h0_t[:, tq * 4:(tq + 1) * 4, :],
                                h_sh[0][tq * 512:(tq + 1) * 512, :]
                                .rearrange("(t p) s -> p t s", p=128))
                        for kc in range(KC):
                            xh0 = atttp.tile([128, NBLK], bf16,
                                             name="xh0", tag="xh0",
                                             bufs=2)
                            eng = nc.sync if kc % 2 == 0 else nc.scalar
                            eng.dma_start(xh0[:], xh_e[:, kc, 0:NBLK])
                            nc.gpsimd.tensor_add(h0_t[:, kc, :],
                                                 h0_t[:, kc, :], xh0[:])
                            nc.gpsimd.tensor_mul(h0sq_t[:, kc, :],
                                                 h0_t[:, kc, :],
                                                 h0_t[:, kc, :])
                    den4 = psDp.tile([128, NBLK], f32, name="den4")
                    sf4 = psSp.tile([128, NBLK], f32, name="sf4",
                                    tag="sbrb", bufs=1)
                    dent4 = attrp.tile([128, NBLK], f32, name="dent4")
                    swb4 = attrp.tile([128, NBLK], bf16, name="swb4")
                    rcpb4 = attrp.tile([128, NBLK], bf16, name="rcpb4")
                    ap_ps = [psAp.tile([128, NBLK], f32, name=f"ap{m}",
                                       bufs=1)
                             for m in range(2)]
                    # self-key q.k products hoisted: they only need the
                    # rope'd q/k, so the DVE does them while the PE runs
                    # the score matmuls; one full-tile mul covers both
                    # heads of an m group
                    qks = []
                    for m in range(2):
                        qk = atttp.tile([128, NBLK], bf16,
                                        name=f"qk{m}",
                                        tag=f"qk{m}", bufs=2)
                        nc.vector.tensor_mul(
                            qk[:, :], qT[m][:, bsl],
                            kT[m][:, S + b * NBLK:S + (b + 1) * NBLK])
                        qks.append(qk)

                    def q_lo(t):
                        """First unmasked q column for key-tile t (the
                        diagonal band is strictly causal: keys t*128+ii
                        only reach q > t*128+ii within the block)."""
                        return max(0, (t - 4 * b) * 128)

                    def s_pair(t):
                        """Score MMs for key-tile t, all 4 heads: two
                        row-tiled concurrent pairs, band-narrowed."""
                        tsl = slice(t * 128, (t + 1) * 128)
                        c0 = q_lo(t)
                        ss = []
                        for m in range(2):
                            for o in (0, 64):
                                hsl = slice(o, o + 64)
                                s_ps = psSp.tile([128, NBLK], f32,
                                                 name="s_ps")
                                nc.tensor.matmul(
                                    s_ps[:, c0:], kT[m][hsl, tsl],
                                    qT[m][hsl,
                                          b * NBLK + c0:(b + 1) * NBLK],
                                    start=True, stop=True,
                                    tile_position=(o, 0))
                                ss.append(s_ps)
                        return ss

                    ss_cur = s_pair(0)
                    for t in range(T):
                        ss_next = s_pair(t + 1) if t + 1 < T else None
                        c0 = q_lo(t)
                        # exp (+ mask on the diagonal band) on ScalarE/DVE
                        ws = []
                        for i, (m, o) in enumerate(
                                ((0, 0), (0, 64), (1, 0), (1, 64))):
                            w_t = attwp.tile([128, NBLK], bf16, name="w_t")
                            nc.scalar.activation(
                                w_t[:, c0:], ss_cur[i][:, c0:], AF.Exp,
                                scale=0.125)
                            if t >= 4 * b:
                                nc.vector.tensor_mul(
                                    w_t[:, c0:], w_t[:, c0:],
                                    masks_t[:, t - 4 * b, c0:])
                            ws.append(w_t)
                        # AV: col-tiled concurrent pairs per m
                        for m in range(2):
                            for io, o in enumerate((0, 64)):
                                nc.tensor.matmul(
                                    ap_ps[m][o:o + 64, c0:],
                                    v_mem[:, t,
                                          m * 128 + o:m * 128 + o + 64],
                                    ws[2 * m + io][:, c0:],
                                    start=(t == 0), stop=(t == T - 1),
                                    tile_position=(0, o))
                        # denominators: concurrent pairs at cols r
                        for m in range(2):
                            for io, o in enumerate((0, 64)):
                                r = 32 * (2 * m + io)
                                nc.tensor.matmul(
                                    den4[r:r + 1, c0:], ones_c[:, 0:1],
                                    ws[2 * m + io][:, c0:],
                                    start=(t == 0), stop=(t == T - 1),
                                    tile_position=(0, r))
                        ss_cur = ss_next

                    # self key: sf matmuls per head, then the whole
                    # denominator chain as full-tile ops (the valid rows
                    # sit at partitions 0/32/64/96; the other partitions
                    # carry garbage that is never read back)
                    heads = [(m, io, o) for m in range(2)
                             for io, o in enumerate((0, 64))]
                    for m, io, o in heads:
                        hsl = slice(o, o + 64)
                        nc.tensor.matmul(
                            sf4[32 * (2 * m + io):32 * (2 * m + io) + 1,
                                :],
                            ones_c[hsl, 0:1], qks[m][hsl, :],
                            start=True, stop=True,
                            tile_position=(o, 32 * (2 * m + io)))
                    nc.scalar.activation(swb4[:], sf4[:], AF.Exp,
                                         scale=0.125)
                    nc.vector.tensor_add(dent4[:], den4[:], swb4[:])
                    lnd = attrp.tile([128, NBLK], f32, name="lnd")
                    nc.scalar.activation(lnd[:], dent4[:], AF.Ln)
                    nc.scalar.activation(rcpb4[:], lnd[:], AF.Exp,
                                         scale=-1.0)
                    # broadcast self_w and 1/den to each head's 64 rows
                    for m in range(2):
                        sb_ps = psSp.tile([128, NBLK], f32, name="sb_ps",
                                          tag="sbrb", bufs=1)
                        rb_ps = psSp.tile([128, NBLK], f32, name="rb_ps",
                                          tag="sbrb", bufs=1)
                        for io, o in enumerate((0, 64)):
                            r = 32 * (2 * m + io)
                            rsl = slice(r, r + 1)
                            nc.tensor.matmul(
                                sb_ps[o:o + 64, :], ones128[rsl, 0:64],
                                swb4[rsl, :], start=True, stop=True,
                                tile_position=(r, o))
                            nc.tensor.matmul(
                                rb_ps[o:o + 64, :], ones128[rsl, 0:64],
                                rcpb4[rsl, :], start=True, stop=True,
                                tile_position=(r, o))
                        # combine: (attn + self_w * vTh) / den
                        t0 = atttp.tile([128, NBLK], bf16, name="t0")
                        nc.vector.tensor_mul(t0[:], vTh[m][:, bsl],
                                             sb_ps[:])
                        t1 = atttp.tile([128, NBLK], bf16, name="t1")
                        nc.vector.tensor_add(t1[:], ap_ps[m][:], t0[:])
                        cmb = cmbp.tile([128, NBLK], bf16, name=f"cmb{m}")
                        nc.vector.tensor_mul(cmb[:], t1[:], rb_ps[:])
                        if m == 0:
                            cmb0 = cmb
                        else:
                            cmb1 = cmb

                    # local o_p: full-D partial from this core's heads
                    # (attention only; the residual joins in the down-proj
                    # partial instead), stored in 512-row quarters
                    for q in range(4):
                        oc_q = ocp.tile([128, 4, NBLK], bf16,
                                        name="oc_q", tag="oc_q", bufs=2)
                        for sub in range(4):
                            md16 = q * 4 + sub
                            ps = psSp.tile([128, NBLK], f32, name="o_ps",
                                           tag="s_ps")
                            nc.tensor.matmul(
                                ps[:],
                                wo_t[:, 0, md16 * 128:(md16 + 1) * 128],
                                cmb0[:], start=True, stop=False)
                            nc.tensor.matmul(
                                ps[:],
                                wo_t[:, 1, md16 * 128:(md16 + 1) * 128],
                                cmb1[:], start=False, stop=True)
                            if sub % 2 == 0:
                                nc.vector.tensor_copy(
                                    oc_q[:, sub, :], ps[:])
                            else:
                                nc.scalar.copy(oc_q[:, sub, :], ps[:])
                        eng = nc.sync if q % 2 == 0 else nc.scalar
                        eng.dma_start(
                            op_dram[b][q * 512:(q + 1) * 512, :].rearrange(
                                "(t p) s -> p t s", p=128),
                            oc_q[:])
                    # one AllReduce per block: every core gets the full
                    # o-sum for this block's columns
                    nc.gpsimd.collective_compute(
                        "AllReduce", mybir.AluOpType.add,
                        replica_groups=rg,
                        ins=[op_dram[b].opt()], outs=[h_sh[b].opt()])
                    if b == 3:
                        # LN2 stats for block 0 (PE/DVE/2 Scalar ops run
                        # in block 3's engine slack / the AR3 window)
                        st0 = psH0p.tile([128, NBLK], f32, name="st0",
                                         tag="st0")
                        for kc in range(KC):
                            nc.tensor.matmul(
                                st0[0:1, :], ones_c[:], h0_t[:, kc, :],
                                start=(kc == 0), stop=(kc == KC - 1),
                                tile_position=(0, 0))
                            nc.tensor.matmul(
                                st0[32:33, :], ones_c[:],
                                h0sq_t[:, kc, :],
                                start=(kc == 0), stop=(kc == KC - 1),
                                tile_position=(0, 32))
                        r160 = atttp.tile([128, NBLK], bf16,
                                          name="r160", tag="r160",
                                          bufs=1)
                        nc.vector.tensor_copy(r160[0:1, :], st0[0:1, :])
                        nc.vector.tensor_copy(r160[32:33, :],
                                              st0[32:33, :])
                        ps0 = psH0p.tile([128, NBLK], f32, name="bc0",
                                         tag="st0")
                        nc.tensor.matmul(ps0[:], invD128[0:1, :],
                                         r160[0:1, :],
                                         start=True, stop=True)
                        mean0 = atttp.tile([128, NBLK], bf16,
                                           name="mean0", tag="mean0",
                                           bufs=1)
                        nc.vector.tensor_copy(mean0[:], ps0[:])
                        ps0b = psH0p.tile([128, NBLK], f32, name="bc0b",
                                          tag="st0")
                        nc.tensor.matmul(ps0b[:], invD128[32:33, :],
                                         r160[32:33, :],
                                         start=True, stop=True)
                        ex20 = atttp.tile([128, NBLK], bf16,
                                          name="ex20", tag="ex20",
                                          bufs=1)
                        nc.vector.tensor_copy(ex20[:], ps0b[:])
                        m20 = atttp.tile([128, NBLK], bf16, name="m20",
                                         tag="m20", bufs=1)
                        nc.vector.tensor_mul(m20[:], mean0[:], mean0[:])
                        nc.vector.tensor_sub(ex20[:], ex20[:], m20[:])
                        nc.scalar.activation(ex20[:], ex20[:], AF.Ln,
                                             bias=eps_c[:])
                        nc.scalar.activation(rstd05[:], ex20[:], AF.Exp,
                                             scale=-0.5)
                        nc.vector.tensor_mul(c05[:], mean0[:],
                                             rstd05[:])
            kqvp.release()

            # second halves of gate/up + the down weight load into the
            # space the attention pools and kqv freed
            wudp = tc.alloc_tile_pool(name="wud", bufs=1)
            wu_h1 = wudp.tile([128, KC, FFL // 2], bf16)
            wg_h2 = wudp.tile([128, KC, FFL // 2], bf16)
            wu_h2 = wudp.tile([128, KC, FFL // 2], bf16)
            wd_t = wudp.tile([128, FFL // 128, D], bf16)

            # ---------- loop 2: LN2 + gated MLP + down + out --------------
            # row-parallel down: each core contracts its own FFL slice of
            # m into a full-D partial which also carries ident8 @ h (the
            # residual + o_sum, scaled 1/8); the per-block ReduceScatter
            # then hands back this core's own rows of h + down_sum — the
            # final output rows, with no separate residual path.
            dp_dram = [dramp.tile([D, NBLK], bf16, name=f"dp_d{b}")
                       for b in range(NB - 1)]
            dp_rs = [dramp.tile([J, NBLK], bf16, name=f"dp_rs{b}")
                     for b in range(NB - 1)]
            dp3_dram = [dramp.tile([D, NBLK // 4], bf16, name=f"dp3_d{i}")
                        for i in range(4)]
            dp3_rs = [dramp.tile([J, NBLK // 4], bf16, name=f"dp3_rs{i}")
                      for i in range(4)]
            with (
                tc.tile_pool(name="hblk", bufs=2) as hblkp,
                tc.tile_pool(name="xh2", bufs=2) as xh2p,
                tc.tile_pool(name="sq5", bufs=2) as sq5p,
                tc.tile_pool(name="sm5", bufs=1) as sm5p,
                tc.tile_pool(name="mloc", bufs=1) as mlocp,
                tc.tile_pool(name="gut", bufs=2) as gutp,
                tc.tile_pool(name="dcp", bufs=2) as dcp,
                tc.tile_pool(name="outt", bufs=1) as outtp,
                tc.tile_pool(name="psG", bufs=2, space="PSUM") as psGp,
                tc.tile_pool(name="psU", bufs=2, space="PSUM") as psUp,
                tc.tile_pool(name="psst5", bufs=1, space="PSUM") as psst5p,
                tc.tile_pool(name="psDn", bufs=2, space="PSUM") as psDnp,
            ):
                def h_load(b):
                    """h_t = o_sum (AllReduced) + xh, raw pre-LN2."""
                    bsl = slice(b * NBLK, (b + 1) * NBLK)
                    h_t = hblkp.tile([128, KC, NBLK], bf16, name="h_t")
                    for tq in range(4):
                        eng = nc.sync if tq % 2 == 0 else nc.scalar
                        eng.dma_start(
                            h_t[:, tq * 4:(tq + 1) * 4, :],
                            h_sh[b][tq * 512:(tq + 1) * 512, :].rearrange(
                                "(t p) s -> p t s", p=128))
                    for kc in range(KC):
                        xh2 = xh2p.tile([128, NBLK], bf16, name="xh2")
                        eng = nc.sync if kc % 2 == 0 else nc.scalar
                        eng.dma_start(xh2[:], xh_e[:, kc, bsl])
                        # adds on the (otherwise idle) gpsimd engine so
                        # the h chain never queues behind DVE work
                        nc.gpsimd.tensor_add(h_t[:, kc, :],
                                             h_t[:, kc, :], xh2[:])
                    return h_t

                def down_block(h_t, m_loc, clo, chi, dpd, dpr):
                    """Down partial over columns [clo, chi) + ident8 @ h;
                    ReduceScatter returns own rows of h + down_sum."""
                    w = chi - clo
                    for q in range(4):
                        dcq = dcp.tile([128, 4, NBLK], bf16, name="dcq")
                        for sub in range(4):
                            md16 = q * 4 + sub
                            ps = psDnp.tile([128, NBLK], f32, name="d_ps")
                            nc.tensor.matmul(
                                ps[:, :w], ident8[:],
                                h_t[:, md16, clo:chi],
                                start=True, stop=False)
                            for fc in range(FFL // 128):
                                nc.tensor.matmul(
                                    ps[:, :w],
                                    wd_t[:, fc,
                                         md16 * 128:(md16 + 1) * 128],
                                    m_loc[:, fc, clo:chi],
                                    start=False,
                                    stop=(fc == FFL // 128 - 1))
                            nc.scalar.copy(dcq[:, sub, :w], ps[:, :w])
                        eng = nc.sync if q % 2 == 0 else nc.scalar
                        eng.dma_start(
                            dpd[q * 512:(q + 1) * 512, :].rearrange(
                                "(t p) s -> p t s", p=128),
                            dcq[:, :, :w])
                    nc.gpsimd.collective_compute(
                        "ReduceScatter", mybir.AluOpType.add,
                        replica_groups=rg,
                        ins=[dpd.opt()], outs=[dpr.opt()])

                def ln2_block(b, h_t):
                    """LN2 stats on raw h; returns broadcast (rstd, c)
                    tiles. gu runs on RAW h with the LN correction folded
                    into its epilogue, so the normalize never sits on the
                    inter-block critical path."""
                    stp = psst5p.tile([128, NBLK], f32, name="st5")
                    for kc in range(KC):
                        sq_t = sq5p.tile([128, NBLK], bf16, name="sq5_t",
                                          bufs=8)
                        nc.scalar.activation(sq_t[:], h_t[:, kc, :],
                                             AF.Square)
                        nc.tensor.matmul(
                            stp[0:1, :], ones_c[:], h_t[:, kc, :],
                            start=(kc == 0), stop=(kc == KC - 1),
                            tile_position=(0, 0))
                        nc.tensor.matmul(
                            stp[32:33, :], ones_c[:], sq_t[:],
                            start=(kc == 0), stop=(kc == KC - 1),
                            tile_position=(0, 32))
                    r16 = sm5p.tile([128, NBLK], bf16, name="r165",
                                    tag="sm5r", bufs=1)
                    nc.vector.tensor_copy(r16[0:1, :], stp[0:1, :])
                    nc.vector.tensor_copy(r16[32:33, :], stp[32:33, :])
                    mean_b = sm5p.tile([128, NBLK], bf16, name="mean5b",
                                       tag="sm5m", bufs=1)
                    ex2_b = sm5p.tile([128, NBLK], bf16, name="ex25b",
                                      tag="sm5e", bufs=1)
                    ps = psst5p.tile([128, NBLK], f32, name="bc_ps",
                                     tag="bc_ps", bufs=1)
                    nc.tensor.matmul(ps[:], invD128[0:1, :], r16[0:1, :],
                                     start=True, stop=True)
                    nc.vector.tensor_copy(mean_b[:], ps[:])
                    ps2 = psst5p.tile([128, NBLK], f32, name="bc_ps2",
                                      tag="bc_ps", bufs=1)
                    nc.tensor.matmul(ps2[:], invD128[32:33, :],
                                     r16[32:33, :],
                                     start=True, stop=True)
                    nc.vector.tensor_copy(ex2_b[:], ps2[:])
                    m2_b = sm5p.tile([128, NBLK], bf16, name="m25b",
                                     tag="sm5m2", bufs=1)
                    nc.vector.tensor_mul(m2_b[:], mean_b[:], mean_b[:])
                    nc.vector.tensor_sub(ex2_b[:], ex2_b[:], m2_b[:])
                    rstd_b = sm5p.tile([128, NBLK], bf16, name="rstd5b",
                                       tag="sm5rs", bufs=2)
                    lnv5 = sm5p.tile([128, NBLK], f32, name="lnv5",
                                     tag="sm5ln", bufs=1)
                    nc.scalar.activation(lnv5[:], ex2_b[:], AF.Ln,
                                         bias=eps_c[:])
                    nc.scalar.activation(rstd_b[:], lnv5[:], AF.Exp,
                                         scale=-0.5)
                    c_bb = sm5p.tile([128, NBLK], bf16, name="c5b",
                                     tag="sm5c", bufs=2)
                    nc.vector.tensor_mul(c_bb[:], mean_b[:], rstd_b[:])
                    return rstd_b, c_bb

                def gu_block(b, h_t, rstd_b, c_bb):
                    """Gate/up on RAW h; LN2 folded into the epilogue:
                    g' = psg*rstd - (c*wsg - bg), same for u."""
                    m_loc = mlocp.tile([128, FFL // 128, NBLK], bf16,
                                       name="m_loc")
                    for mf in range(FFL // 128):
                        if mf < FFL // 256:
                            wgs, wus, mfl = wg_h1, wu_h1, mf
                        else:
                            wgs, wus, mfl = wg_h2, wu_h2, mf - FFL // 256
                        psg = psGp.tile([128, NBLK], f32, name="g_ps")
                        psu = psUp.tile([128, NBLK], f32, name="u_ps")
                        for kc in range(KC):
                            nc.tensor.matmul(
                                psg[:],
                                wgs[:, kc, mfl * 128:(mfl + 1) * 128],
                                h_t[:, kc, :],
                                start=(kc == 0), stop=(kc == KC - 1))
                            nc.tensor.matmul(
                                psu[:],
                                wus[:, kc, mfl * 128:(mfl + 1) * 128],
                                h_t[:, kc, :],
                                start=(kc == 0), stop=(kc == KC - 1))
                        cwg = gutp.tile([128, NBLK], bf16, name="cwg")
                        nc.vector.tensor_scalar(
                            out=cwg[:], in0=c_bb[:],
                            scalar1=wsg_t[:, mf:mf + 1],
                            scalar2=bg_t[:, mf:mf + 1],
                            op0=Alu.mult, op1=Alu.subtract)
                        g_t = gutp.tile([128, NBLK], bf16, name="g_t")
                        nc.vector.tensor_mul(g_t[:], psg[:], rstd_b[:])
                        nc.vector.tensor_sub(g_t[:], g_t[:], cwg[:])
                        cwu = gutp.tile([128, NBLK], bf16, name="cwu")
                        nc.vector.tensor_scalar(
                            out=cwu[:], in0=c_bb[:],
                            scalar1=wsu_t[:, mf:mf + 1],
                            scalar2=bu_t[:, mf:mf + 1],
                            op0=Alu.mult, op1=Alu.subtract)
                        u_t = gutp.tile([128, NBLK], bf16, name="u_t")
                        nc.vector.tensor_mul(u_t[:], psu[:], rstd_b[:])
                        nc.vector.tensor_sub(u_t[:], u_t[:], cwu[:])
                        sg = gutp.tile([128, NBLK], bf16, name="sg")
                        nc.scalar.activation(sg[:], g_t[:], AF.Sigmoid)
                        silu = gutp.tile([128, NBLK], bf16, name="silu")
                        nc.vector.tensor_mul(silu[:], g_t[:], sg[:])
                        nc.vector.tensor_mul(m_loc[:, mf, :], silu[:],
                                             u_t[:])
                    return m_loc

                def out_block(b, clo, chi, dpr):
                    """Own jsl rows of h + down_sum -> out (f32).

                    The rsd load waits on the ReduceScatter, so it
                    rides the gpsimd queue: on sync/scalar it would
                    head-of-line block the next block's h loads and
                    Square stream behind that wait."""
                    w = chi - clo
                    rsd = outtp.tile([128, MD, NBLK], bf16, name="rsd")
                    nc.gpsimd.dma_start(
                        rsd[:, :, :w], dpr[:, :].rearrange(
                            "(t p) s -> p t s", p=128))
                    for md in range(MD):
                        ot = outtp.tile([128, NBLK], f32, name="ot")
                        nc.vector.tensor_copy(ot[:, :w], rsd[:, md, :w])
                        nc.gpsimd.dma_start(
                            out_e[md, :, b * NBLK + clo:b * NBLK + chi],
                            ot[:, :w])

                # per-block pipeline; the final block's down/RS/out is
                # split into column halves to shorten the exposed tail
                h_cur = h0_t
                st_cur = (rstd05, c05)
                # weights: wu first half first (gu's first up-matmuls
                # need it almost immediately)
                nc.scalar.dma_start(wu_h1[:], wu_e[:, :, :FFL // 2])
                nc.sync.dma_start(wg_h2[:], wg_e[:, :, FFL // 2:])
                nc.scalar.dma_start(wu_h2[:], wu_e[:, :, FFL // 2:])
                nc.sync.dma_start(wd_t[:, :FFL // 256, :],
                                  wd_e[:, :FFL // 256, :])
                nc.scalar.dma_start(wd_t[:, FFL // 256:, :],
                                    wd_e[:, FFL // 256:, :])
                # ln2(b+1)'s serial Square/stats chain is emitted before
                # gu(b)/down(b) so it overlaps their PE streams
                for b in range(NB):
                    if b + 1 < NB:
                        h_next = h_load(b + 1)
                        st_next = ln2_block(b + 1, h_next)
                    else:
                        h_next = st_next = None
                    ml = gu_block(b, h_cur, *st_cur)
                    if b < NB - 1:
                        down_block(h_cur, ml, 0, NBLK,
                                   dp_dram[b], dp_rs[b])
                        out_block(b, 0, NBLK, dp_rs[b])
                    else:
                        for qq in range(4):
                            qlo = qq * (NBLK // 4)
                            qhi = (qq + 1) * (NBLK // 4)
                            down_block(h_cur, ml, qlo, qhi,
                                       dp3_dram[qq], dp3_rs[qq])
                            out_block(b, qlo, qhi, dp3_rs[qq])
                    h_cur, st_cur = h_next, st_next
            wudp.release()
            h0p.release()
            wguh1p.release()

    return nc


# ---------------------------------------------------------------------------
# Host side
# ---------------------------------------------------------------------------

def _chunkT(a):
    """[R, D] -> [128, D//128, R] view for lhsT/rhs chunk layout.

    Result[p, kc, r] = a[r, kc*128 + p].
    """
    R, Dd = a.shape
    return np.ascontiguousarray(
        a.reshape(R, Dd // 128, 128).transpose(2, 1, 0))


def prepare_inputs(hidden_states, memory, position_ids,
                   ln1_w, ln1_b, ln2_w, ln2_b,
                   Wq, Wk, Wv, Wo, Wg, Wu, Wd, S):
    """Build the 8 per-core in_maps (numpy host prep)."""
    f32 = np.float32
    hid = np.asarray(hidden_states, f32)[0]       # [S, D]
    mem = np.asarray(memory, f32)[0]
    pos = np.asarray(position_ids)[0].astype(np.float64)

    Wq1 = np.asarray(Wq, f32) * np.asarray(ln1_w, f32)[None, :]
    Wk1 = np.asarray(Wk, f32) * np.asarray(ln1_w, f32)[None, :]
    Wv1 = np.asarray(Wv, f32) * np.asarray(ln1_w, f32)[None, :]
    bq = np.asarray(Wq, f32) @ np.asarray(ln1_b, f32)
    bk = np.asarray(Wk, f32) @ np.asarray(ln1_b, f32)
    bv = np.asarray(Wv, f32) @ np.asarray(ln1_b, f32)
    Wg2 = np.asarray(Wg, f32) * np.asarray(ln2_w, f32)[None, :]
    Wu2 = np.asarray(Wu, f32) * np.asarray(ln2_w, f32)[None, :]
    bg = np.asarray(Wg, f32) @ np.asarray(ln2_b, f32)
    bu = np.asarray(Wu, f32) @ np.asarray(ln2_b, f32)
    Wo_ = np.asarray(Wo, f32)
    Wd_ = np.asarray(Wd, f32)

    # x^T chunk layouts (shared by all cores)
    xm = _chunkT(mem).astype(BF16)                # [128, KC, S]
    xh = _chunkT(hid).astype(BF16)

    # rope tables [128, 2S], row pattern period 16
    inv = BASE ** (-(np.arange(8, dtype=np.float64) * 2) / RD)
    t = pos[:, None] * inv[None, :]               # [S, 8]
    cos8 = np.cos(t).T                            # [8, S]
    sin8 = np.sin(t).T
    cos16 = np.concatenate([cos8, cos8], 0)       # [16, S]
    sin16 = np.concatenate([-sin8, sin8], 0)
    cosf = np.tile(np.concatenate([cos16, cos16], 1), (8, 1))  # [128, 2S]
    sinf = np.tile(np.concatenate([sin16, sin16], 1), (8, 1))
    cosf = cosf.astype(BF16)
    sinf = sinf.astype(BF16)

    ident8 = (np.eye(128) * 0.125).astype(BF16)

    # strict-causal masks for the 4 diagonal-band offsets
    ii = np.arange(128)[:, None]
    jj = np.arange(NBLK)[None, :]
    masks = np.stack(
        [(ii + 128 * o < jj) for o in range(4)], 1).astype(BF16)  # [128,4,512]

    in_maps = []
    for c in range(N_CORES):
        jsl = slice(c * J, (c + 1) * J)
        fsl = slice(c * FFL, (c + 1) * FFL)
        wq_c = Wq1[jsl]                            # [J, D]
        wk_c = Wk1[jsl]
        wv_c = Wv1[jsl]
        im = {
            "xm": xm, "xh": xh,
            "ident8": ident8,
            "wq": _chunkT(wq_c).astype(BF16),
            "wk": _chunkT(wk_c).astype(BF16),
            "wv": _chunkT(wv_c).astype(BF16),
            "wo_p": _chunkT(Wo_[:, jsl]).astype(BF16),
            "wg": _chunkT(Wg2[fsl]).astype(BF16),
            "wu": _chunkT(Wu2[fsl]).astype(BF16),
            "wd": _chunkT(Wd_[:, fsl]).astype(BF16),
            "wsq": np.ascontiguousarray(
                wq_c.sum(1).reshape(MD, 128).T).astype(f32),
            "wsk": np.ascontiguousarray(
                wk_c.sum(1).reshape(MD, 128).T).astype(f32),
            "wsvc": np.ascontiguousarray(
                wv_c.sum(1).reshape(MD, 128).T).astype(f32),
            "bq": np.ascontiguousarray(
                bq[jsl].reshape(MD, 128).T).astype(f32),
            "bk": np.ascontiguousarray(
                bk[jsl].reshape(MD, 128).T).astype(f32),
            "bvc": np.ascontiguousarray(
                bv[jsl].reshape(MD, 128).T).astype(f32),
            "wsv_row": wv_c.sum(1)[None, :].astype(f32),
            "bv_row": bv[jsl][None, :].astype(f32),
            "bg": np.ascontiguousarray(
                bg[fsl].reshape(FFL // 128, 128).T).astype(f32),
            "bu": np.ascontiguousarray(
                bu[fsl].reshape(FFL // 128, 128).T).astype(f32),
            "wsg": np.ascontiguousarray(
                Wg2[fsl].sum(1).reshape(FFL // 128, 128).T).astype(f32),
            "wsu": np.ascontiguousarray(
                Wu2[fsl].sum(1).reshape(FFL // 128, 128).T).astype(f32),
            "rope_cos": cosf, "rope_sinsg": sinf,
            "masks": masks,
        }
        in_maps.append(im)
    return in_maps


def assemble_output(results, S):
    outT = np.concatenate(
        [np.asarray(results[c]["out"]).reshape(J, S)
         for c in range(N_CORES)], 0)              # [D, S]
    return np.ascontiguousarray(outT.T).reshape(1, S, D).astype(np.float32)


_GRAPH_CACHE = {}


def get_graph(S):
    if S not in _GRAPH_CACHE:
        _GRAPH_CACHE[S] = build_graph(S)
    return _GRAPH_CACHE[S]


def kernel(hidden_states, memory, attention_mask, position_ids,
           ln1_w, ln1_b, ln2_w, ln2_b, Wq, Wk, Wv, Wo, Wg, Wu, Wd):
    from concourse.bass_utils import run_bass_kernel_spmd

    S = np.asarray(hidden_states).shape[1]
    in_maps = prepare_inputs(
        hidden_states, memory, position_ids, ln1_w, ln1_b, ln2_w, ln2_b,
        Wq, Wk, Wv, Wo, Wg, Wu, Wd, S)
    nc = get_graph(S)
    res = run_bass_kernel_spmd(nc, in_maps, core_ids=list(range(N_CORES)))
    return assemble_output(res.results, S)



# revision 37
# speedup vs baseline: 1.0184x; 1.0184x over previous
"""Trainium2 Bass kernel for nn_ArcDecoderLayer (sparse_attention).

Self-contained: takes FULL unsharded inputs, shards across 8 NeuronCores
(head-parallel attention, row-parallel o_proj with AllReduce, FF-parallel
MLP with AllGather of the intermediate), returns the FULL output.

v2 layout/schedule:
- LN1/LN2 statistics as concurrent col-tiled (sum, sumsq) matmul pairs;
  raw sums broadcast with 1/D folded into the broadcast constant so all
  stats math runs partition-aligned on full tiles.
- Attention: per-key-tile waves; score MMs row-tiled concurrent pairs,
  AV MMs col-tiled concurrent pairs, denominator MMs concurrent pairs,
  software-pipelined one key-tile ahead of the ScalarE exp; diagonal
  band tiles narrowed to their unmasked column range; packed full-width
  RoPE.
- o_proj computed locally from this core's heads into a full-D partial;
  per-block ReduceScatter(+AllGather) gives the residual rows and the
  full h sum (no attn/o AllGathers).
- MLP: gate/up per 512-block; row-parallel down-proj straight from the
  SBUF-resident m slice into a full-D partial, ReduceScattered back (no
  m AllGather, no 33MB of gathered-m reads).
All matmul compute bf16 with f32 PSUM accumulation.
"""

import sys
import types

sys.path.insert(0, "/opt/trn_rl_repo")

# ---- shim antenv.axon_hooks so trace=True profiling works in this image ----
if "antenv.axon_hooks" not in sys.modules:
    _hook_mod = types.ModuleType("antenv.axon_hooks")
    _hook_state = {"hook": None}

    def _set_hook(h):
        _hook_state["hook"] = h

    def _get_hook():
        return _hook_state["hook"]

    _hook_mod.set_axon_ntff_profile_hook = _set_hook
    _hook_mod.get_axon_ntff_profile_hook = _get_hook
    sys.modules["antenv.axon_hooks"] = _hook_mod
    try:
        import antenv

        antenv.axon_hooks = _hook_mod
        from trn_agent_boot.trn_boot import _ntff_profile_via_ctypes

        _set_hook(_ntff_profile_via_ctypes("/opt/axon/libaxon_pjrt.so"))
    except Exception:
        pass

import numpy as np
import ml_dtypes

import concourse.bass as bass
import concourse.mybir as mybir
import concourse.tile as tile
from concourse import library_config
from concourse.vector_clock import ScopedClock

BF16 = ml_dtypes.bfloat16

N_CORES = 8
D = 2048
FF = 8192
H = 32
DH = 64
RD = 16
EPS = 1e-5
BASE = 10000.0

J = D // N_CORES        # 256 head-dims per core (4 heads)
FFL = FF // N_CORES     # 1024 ff dims per core
KC = D // 128           # 16 contraction chunks
NBLK = 512              # lq block width
MD = J // 128           # 2 output Mtiles per core for down/out


WAIT_LIMITS = {"InstNoOp": 1, "InstDrain": 1, "InstEventSemaphore": 1}
DEFAULT_WAIT_LIMIT = 1


class PatchedTC(tile.TileContext):
    """TileContext patched for this walrus build, which rejects instructions
    carrying more than a couple of sync wait commands: excess waits are
    split onto injected same-engine nops just before the instruction."""

    _wsplit_n = 0

    def _split_excess_waits(self, ordered):
        for bb, insts in ordered.items():
            out = []
            for inst in insts:
                si = inst.sync_info
                waits = list(si.on_wait) if si and si.on_wait else []
                lim = WAIT_LIMITS.get(type(inst).__name__,
                                      DEFAULT_WAIT_LIMIT)
                if len(waits) > lim:
                    for w in waits[:-lim]:
                        nop = mybir.InstNoOp(
                            name=f"I-wsplit-{PatchedTC._wsplit_n}",
                            ins=[], outs=[], engine=inst.engine,
                            nofuse=True)
                        PatchedTC._wsplit_n += 1
                        nop.sync_info = mybir.SyncInfo(
                            on_wait=[w], on_update=[])
                        out.append(nop)
                    inst.sync_info = mybir.SyncInfo(
                        on_wait=waits[-lim:],
                        on_update=list(si.on_update or []))
                out.append(inst)
            ordered[bb] = out

    def _lower_ordered_insts(self, ordered):
        self._split_excess_waits(ordered)
        return super()._lower_ordered_insts(ordered)

    def _drain_and_barrier(self, tick_clock, wait_clock):
        nc = self.nc
        probe = nc.sync.nop(nofuse=True, hint="tail_wait_probe")
        wait_clock.add_sem_waits(
            probe.ins, ScopedClock({None: tick_clock.global_clock})
        )
        waits = list(probe.ins.sync_info.on_wait or [])
        probe.ins.sync_info.on_wait = waits[:1]
        for i in range(1, len(waits)):
            n = nc.sync.nop(nofuse=True, hint=f"tail_wait_{i}")
            n.ins.sync_info = mybir.SyncInfo(on_wait=[waits[i]], on_update=[])
        nc.sync.drain()
        nc.all_engine_barrier()
        assert self.sems is not None
        popped = nc._tile_sem_poison_stack.pop()
        assert popped is self._sem_poison
        nc.clear_and_free_semaphores(list(self.sems.allocated().values()))
        nc.all_engine_barrier()


def build_graph(S):
    """Build the SPMD 8-core graph for sequence length S (multiple of 512)."""
    dt = mybir.dt
    f32, bf16 = dt.float32, dt.bfloat16
    AF = mybir.ActivationFunctionType
    Alu = mybir.AluOpType
    NB = S // NBLK          # lq blocks (4)
    LT = S // 128           # 128-wide l tiles per part
    S2 = 2 * S

    nc = bass.Bass()
    P = nc.declare_dram_parameter

    xm_e = P("xm", [128, KC, S], bf16, isOutput=False)
    xh_e = P("xh", [128, KC, S], bf16, isOutput=False)
    ident8_e = P("ident8", [128, 128], bf16, isOutput=False)
    wq_e = P("wq", [128, KC, J], bf16, isOutput=False)
    wk_e = P("wk", [128, KC, J], bf16, isOutput=False)
    wv_e = P("wv", [128, KC, J], bf16, isOutput=False)
    wo_e = P("wo_p", [128, MD, D], bf16, isOutput=False)
    wg_e = P("wg", [128, KC, FFL], bf16, isOutput=False)
    wu_e = P("wu", [128, KC, FFL], bf16, isOutput=False)
    wd_e = P("wd", [128, FFL // 128, D], bf16, isOutput=False)
    # column (per-partition) weight rowsums + biases for q/k/vTh epilogues
    wsq_e = P("wsq", [128, 2], f32, isOutput=False)
    wsk_e = P("wsk", [128, 2], f32, isOutput=False)
    wsvc_e = P("wsvc", [128, 2], f32, isOutput=False)   # for vT_h epilogue
    bq_e = P("bq", [128, 2], f32, isOutput=False)
    bk_e = P("bk", [128, 2], f32, isOutput=False)
    bvc_e = P("bvc", [128, 2], f32, isOutput=False)
    # row layouts for v_mem epilogue
    wsv_e = P("wsv_row", [1, J], f32, isOutput=False)
    bv_e = P("bv_row", [1, J], f32, isOutput=False)
    bg_e = P("bg", [128, FFL // 128], f32, isOutput=False)
    bu_e = P("bu", [128, FFL // 128], f32, isOutput=False)
    wsg_e = P("wsg", [128, FFL // 128], f32, isOutput=False)
    wsu_e = P("wsu", [128, FFL // 128], f32, isOutput=False)
    ropec_e = P("rope_cos", [128, S2], bf16, isOutput=False)
    ropes_e = P("rope_sinsg", [128, S2], bf16, isOutput=False)
    masks_e = P("masks", [128, 4, NBLK], bf16, isOutput=False)
    out_e = P("out", [MD, 128, S], f32, isOutput=True)

    rg = [list(range(N_CORES))]

    with PatchedTC(nc) as tc:
        with (
            tc.tile_pool(name="const", bufs=1) as constp,
            tc.tile_pool(name="dram", bufs=1, space="DRAM") as dramp,
            tc.tile_pool(name="dsh", bufs=1, space="DRAM") as dshp,
        ):
            # first half of the gate weight lives below kqvp on the
            # pool stack so it can prefetch during attention and survive
            # into the MLP loop (releases stay LIFO)
            wguh1p = tc.alloc_tile_pool(name="wguh1", bufs=1)
            wg_h1 = wguh1p.tile([128, KC, FFL // 2], bf16)
            # block-0 h + LN2 stats, precomputed inside attention so the
            # MLP can start the moment attention drains
            h0p = tc.alloc_tile_pool(name="h0", bufs=1)
            h0_t = h0p.tile([128, KC, NBLK], bf16)
            rstd05 = h0p.tile([128, NBLK], bf16)
            c05 = h0p.tile([128, NBLK], bf16)
            kqvp = tc.alloc_tile_pool(name="kqv", bufs=1)
            ones_c = constp.tile([128, 1], bf16)
            nc.vector.memset(ones_c[:], 1.0)
            ones128 = constp.tile([128, 128], bf16)
            nc.vector.memset(ones128[:], 1.0)
            invD128 = constp.tile([128, 128], bf16)
            nc.vector.memset(invD128[:], 1.0 / D)
            eps_c = constp.tile([128, 1], f32)
            nc.vector.memset(eps_c[:], EPS)
            onesf = constp.tile([1, 128], f32)
            nc.vector.memset(onesf[:], 1.0)
            ident8 = constp.tile([128, 128], bf16)
            nc.gpsimd.dma_start(ident8[:], ident8_e[:])

            def bcast_rows(dst, src_row, width, pspool, ones_row):
                """dst[0:128, :width] = src_row[0, :width] via K=1 matmuls
                (partition_broadcast is not encodable by this walrus)."""
                for i in range(0, width, NBLK):
                    w = min(NBLK, width - i)
                    ps = pspool.tile([128, NBLK], f32, name="bc_ps",
                                     tag="bc_ps", bufs=1)
                    nc.tensor.matmul(ps[:, :w], ones_row[0:1, :],
                                     src_row[0:1, i:i + w],
                                     start=True, stop=True)
                    nc.vector.tensor_copy(dst[:, i:i + w], ps[:, :w])

            wsvb = constp.tile([128, J], f32)
            wsv_row = constp.tile([1, J], f32)
            nc.sync.dma_start(wsv_row[:], wsv_e[:])
            bvb = constp.tile([128, J], f32)
            bv_row = constp.tile([1, J], f32)
            nc.sync.dma_start(bv_row[:], bv_e[:])
            wsq_t = constp.tile([128, 2], f32)
            nc.sync.dma_start(wsq_t[:], wsq_e[:])
            wsk_t = constp.tile([128, 2], f32)
            nc.sync.dma_start(wsk_t[:], wsk_e[:])
            wsvc_t = constp.tile([128, 2], f32)
            nc.sync.dma_start(wsvc_t[:], wsvc_e[:])
            bq_t = constp.tile([128, 2], f32)
            nc.sync.dma_start(bq_t[:], bq_e[:])
            bk_t = constp.tile([128, 2], f32)
            nc.sync.dma_start(bk_t[:], bk_e[:])
            bvc_t = constp.tile([128, 2], f32)
            nc.sync.dma_start(bvc_t[:], bvc_e[:])
            bg_t = constp.tile([128, FFL // 128], f32)
            nc.sync.dma_start(bg_t[:], bg_e[:])
            bu_t = constp.tile([128, FFL // 128], f32)
            nc.sync.dma_start(bu_t[:], bu_e[:])
            wsg_t = constp.tile([128, FFL // 128], f32)
            nc.sync.dma_start(wsg_t[:], wsg_e[:])
            wsu_t = constp.tile([128, FFL // 128], f32)
            nc.sync.dma_start(wsu_t[:], wsu_e[:])

            # persistent QKV outputs
            kT = [kqvp.tile([128, S2], bf16, name=f"kT{m}") for m in range(2)]
            qT = [kqvp.tile([128, S], bf16, name=f"qT{m}") for m in range(2)]
            vTh = [kqvp.tile([128, S], bf16, name=f"vTh{m}") for m in range(2)]
            v_mem = kqvp.tile([128, LT, J], bf16)

            # v_mem epilogue needs column-layout stats of the mem part
            rstd_col_mem = constp.tile([128, LT], f32)
            c_col_mem = constp.tile([128, LT], f32)

            # rope tables (persistent through phase 1)
            rope_loaded = [False]

            def load_rope_tables():
                if not rope_loaded[0]:
                    nc.sync.dma_start(cos_t[:], ropec_e[:, 0:S])
                    nc.sync.dma_start(sin_t[:], ropes_e[:, 0:S])
                    rope_loaded[0] = True

            def rope_packed(slices, name):
                """Apply partial rotary to up to 8 (tile, col_lo) 16-row
                rotary groups at once, packed across all 128 partitions.
                Each slice is (tile, row_lo, col_lo); processes
                tile[row_lo:row_lo+16, col_lo:col_lo+S]. The cos/sin
                tables have the same 16-row pattern on every group, and
                positions repeat across both S-halves."""
                pk = ropep.tile([128, S], bf16, name=f"pk_{name}",
                                tag="rope_pk", bufs=1)
                sw = ropep.tile([128, S], bf16, name=f"sw_{name}",
                                tag="rope_sw", bufs=1)
                for i, (t, rlo, clo) in enumerate(slices):
                    csl = slice(clo, clo + S)
                    eng = nc.sync if i % 2 == 0 else nc.scalar
                    eng.dma_start(pk[16 * i:16 * i + 16, :],
                                  t[rlo:rlo + 16, csl])
                    eng.dma_start(sw[16 * i:16 * i + 8, :],
                                  t[rlo + 8:rlo + 16, csl])
                    eng.dma_start(sw[16 * i + 8:16 * i + 16, :],
                                  t[rlo:rlo + 8, csl])
                n = 16 * len(slices)
                nc.vector.tensor_mul(pk[:n, :], pk[:n, :], cos_t[:n, :])
                nc.vector.tensor_mul(sw[:n, :], sw[:n, :], sin_t[:n, :])
                nc.vector.tensor_add(pk[:n, :], pk[:n, :], sw[:n, :])
                for i, (t, rlo, clo) in enumerate(slices):
                    csl = slice(clo, clo + S)
                    eng = nc.sync if i % 2 == 0 else nc.scalar
                    eng.dma_start(t[rlo:rlo + 16, csl],
                                  pk[16 * i:16 * i + 16, :])

            # ---------- LN1 stats (row-major, col-tiled pairs) -------------
            def stats_nb(x_nb, nb, sqp, psp, row16):
                """Per 512-block LN sums: sum -> stp[0:1] ((0,0)),
                sumsq -> stp[32:33] ((0,32)): concurrent col-tiled pairs
                at different partitions of one PSUM bank."""
                nsl = slice(nb * NBLK, (nb + 1) * NBLK)
                stp = psp.tile([128, NBLK], f32, name="stp")
                for kc in range(KC):
                    sq_t = sqp.tile([128, NBLK], bf16, name="sq_t")
                    nc.scalar.activation(sq_t[:], x_nb[:, kc, :],
                                         AF.Square)
                    nc.tensor.matmul(
                        stp[0:1, :], ones_c[:], x_nb[:, kc, :],
                        start=(kc == 0), stop=(kc == KC - 1),
                        tile_position=(0, 0))
                    nc.tensor.matmul(
                        stp[32:33, :], ones_c[:], sq_t[:],
                        start=(kc == 0), stop=(kc == KC - 1),
                        tile_position=(0, 32))
                nc.vector.tensor_copy(row16[0:1, nsl], stp[0:1, :])
                nc.vector.tensor_copy(row16[32:33, nsl], stp[32:33, :])

            def ln_stats_fin(row16, psp, rowp, part_name, want_col):
                """Broadcast the raw sums (1/D folded into the broadcast
                constant) and finish the stats math on full [128, S]
                tiles. Returns (rstd_b, c_b)."""
                mean_b = rowp.tile([128, S], bf16, name="mean_b",
                                   tag="meanb")
                ex2_b = rowp.tile([128, S], bf16, name="ex2_b", tag="ex2b")
                for i in range(0, S, NBLK):
                    isl = slice(i, i + NBLK)
                    ps = psp.tile([128, NBLK], f32, name="bc_ps",
                                  tag="bc_ps", bufs=1)
                    nc.tensor.matmul(ps[:], invD128[0:1, :],
                                     row16[0:1, isl],
                                     start=True, stop=True)
                    nc.vector.tensor_copy(mean_b[:, isl], ps[:])
                    ps2 = psp.tile([128, NBLK], f32, name="bc_ps2",
                                   tag="bc_ps", bufs=1)
                    nc.tensor.matmul(ps2[:], invD128[32:33, :],
                                     row16[32:33, isl],
                                     start=True, stop=True)
                    nc.vector.tensor_copy(ex2_b[:, isl], ps2[:])
                m2_b = rowp.tile([128, S], bf16, name="m2_b", tag="m2b")
                nc.vector.tensor_mul(m2_b[:], mean_b[:], mean_b[:])
                nc.vector.tensor_sub(ex2_b[:], ex2_b[:], m2_b[:])
                rstd_b = rowp.tile([128, S], bf16, name="rstd_b",
                                   tag="rstdb")
                nc.scalar.activation(ex2_b[:], ex2_b[:], AF.Ln,
                                     bias=eps_c[:])
                nc.scalar.activation(rstd_b[:], ex2_b[:], AF.Exp,
                                     scale=-0.5)
                c_b = rowp.tile([128, S], bf16, name="c_b", tag="cb")
                nc.vector.tensor_mul(c_b[:], mean_b[:], rstd_b[:])
                if want_col:
                    # round-trip on the vector queue so the xh loads on
                    # sync/scalar are not stuck behind this stats chain
                    for nm, row, col in (("rstd", rstd_b, rstd_col_mem),
                                         ("c", c_b, c_col_mem)):
                        dr = dramp.tile([S], bf16,
                                        name=f"st_{nm}_{part_name}")
                        nc.gpsimd.dma_start(
                            dr[:].rearrange("(o a) -> o a", o=1),
                            row[0:1, :])
                        col16 = rowp.tile([128, LT], bf16,
                                          name=f"c16_{nm}", tag="col16")
                        nc.gpsimd.dma_start(
                            col16[:],
                            dr[:].rearrange("(t p) -> p t", p=128))
                        nc.vector.tensor_copy(col[:], col16[:])
                return rstd_b, c_b

            def fin_nb(row16, nb, scrp, psp):
                """Per-512-block stats finalize: broadcast raw sums and
                produce (rstd, c) tiles for just these columns, so the
                epilogues + rope for block nb can run while later blocks
                still project."""
                nsl = slice(nb * NBLK, (nb + 1) * NBLK)
                mean5 = scrp.tile([128, NBLK], bf16, name="mean5",
                                  tag="f_mean", bufs=2)
                ex25 = scrp.tile([128, NBLK], bf16, name="ex25",
                                 tag="f_ex2", bufs=2)
                ps = psp.tile([128, NBLK], f32, name="bc_ps",
                              tag="bc_ps", bufs=1)
                nc.tensor.matmul(ps[:], invD128[0:1, :], row16[0:1, nsl],
                                 start=True, stop=True)
                nc.vector.tensor_copy(mean5[:], ps[:])
                ps2 = psp.tile([128, NBLK], f32, name="bc_ps2",
                               tag="bc_ps", bufs=1)
                nc.tensor.matmul(ps2[:], invD128[32:33, :],
                                 row16[32:33, nsl],
                                 start=True, stop=True)
                nc.vector.tensor_copy(ex25[:], ps2[:])
                m25 = scrp.tile([128, NBLK], bf16, name="m25",
                                tag="f_m2", bufs=2)
                nc.vector.tensor_mul(m25[:], mean5[:], mean5[:])
                nc.vector.tensor_sub(ex25[:], ex25[:], m25[:])
                nc.scalar.activation(ex25[:], ex25[:], AF.Ln,
                                     bias=eps_c[:])
                rstd5 = scrp.tile([128, NBLK], bf16, name="rstd5",
                                  tag="f_rstd", bufs=2)
                nc.scalar.activation(rstd5[:], ex25[:], AF.Exp,
                                     scale=-0.5)
                c5 = scrp.tile([128, NBLK], bf16, name="c5",
                               tag="f_c", bufs=2)
                nc.vector.tensor_mul(c5[:], mean5[:], rstd5[:])
                return rstd5, c5

            def proj_epi_nb(dst, dst_off, nb, rstd5, c5, ws_t, b_t,
                            scrp):
                """LN epilogue for one 512-block: d = d*rstd-(c*ws-b)."""
                for m in range(2):
                    d = dst[m][:, dst_off + nb * NBLK:
                               dst_off + (nb + 1) * NBLK]
                    cw = scrp.tile([128, NBLK], bf16, name="cw_nb",
                                   tag="cw_nb", bufs=2)
                    nc.vector.tensor_scalar(
                        out=cw[:], in0=c5[:],
                        scalar1=ws_t[:, m:m + 1],
                        scalar2=b_t[:, m:m + 1],
                        op0=Alu.mult, op1=Alu.subtract)
                    nc.vector.tensor_mul(d, d, rstd5[:])
                    nc.vector.tensor_sub(d, d, cw[:])

            def rope_packed_nb(slices, name, nb):
                """rope_packed restricted to one 512-column block."""
                nsl = slice(nb * NBLK, (nb + 1) * NBLK)
                pk = ropep.tile([128, NBLK], bf16, name=f"pk_{name}",
                                tag="rope_pk", bufs=2)
                sw = ropep.tile([128, NBLK], bf16, name=f"sw_{name}",
                                tag="rope_sw", bufs=2)
                for i, (t, rlo, clo) in enumerate(slices):
                    csl = slice(clo + nb * NBLK, clo + (nb + 1) * NBLK)
                    eng = nc.sync if i % 2 == 0 else nc.scalar
                    eng.dma_start(pk[16 * i:16 * i + 16, :],
                                  t[rlo:rlo + 16, csl])
                    eng.dma_start(sw[16 * i:16 * i + 8, :],
                                  t[rlo + 8:rlo + 16, csl])
                    eng.dma_start(sw[16 * i + 8:16 * i + 16, :],
                                  t[rlo:rlo + 8, csl])
                n = 16 * len(slices)
                nc.vector.tensor_mul(pk[:n, :], pk[:n, :],
                                     cos_t[:n, nsl])
                nc.vector.tensor_mul(sw[:n, :], sw[:n, :],
                                     sin_t[:n, nsl])
                nc.vector.tensor_add(pk[:n, :], pk[:n, :], sw[:n, :])
                for i, (t, rlo, clo) in enumerate(slices):
                    csl = slice(clo + nb * NBLK, clo + (nb + 1) * NBLK)
                    eng = nc.sync if i % 2 == 0 else nc.scalar
                    eng.dma_start(t[rlo:rlo + 16, csl],
                                  pk[16 * i:16 * i + 16, :])

            def proj_raw_nb(wt, dst, dst_off, x_nb, nb, psp):
                """Raw projection matmuls for one 512-column block,
                copied to dst bf16 (no LN dependency)."""
                for m in range(2):
                    ps = psp.tile([128, NBLK], f32, name="proj_ps")
                    for kc in range(KC):
                        nc.tensor.matmul(
                            ps[:],
                            wt[:, kc, m * 128:(m + 1) * 128],
                            x_nb[:, kc, :],
                            start=(kc == 0), stop=(kc == KC - 1))
                    d = dst[m][:, dst_off + nb * NBLK:
                               dst_off + (nb + 1) * NBLK]
                    nc.vector.tensor_copy(d, ps[:])

            def proj_epi(dst, dst_off, rstd_b, c_b, ws_t, b_t, scrp):
                """LN epilogue in place: d = d*rstd - (c*ws - bias)."""
                for m in range(2):
                    for nb in range(NB):
                        sl = slice(nb * NBLK, (nb + 1) * NBLK)
                        d = dst[m][:, dst_off + nb * NBLK:
                                   dst_off + (nb + 1) * NBLK]
                        cw = scrp.tile([128, NBLK], bf16, name="cw_nb",
                                       tag="cw_nb", bufs=2)
                        nc.vector.tensor_scalar(
                            out=cw[:], in0=c_b[:, sl],
                            scalar1=ws_t[:, m:m + 1],
                            scalar2=b_t[:, m:m + 1],
                            op0=Alu.mult, op1=Alu.subtract)
                        nc.vector.tensor_mul(d, d, rstd_b[:, sl])
                        nc.vector.tensor_sub(d, d, cw[:])

            with (
                tc.tile_pool(name="wqkv", bufs=1) as wqkvp,
                tc.tile_pool(name="psq", bufs=2, space="PSUM") as psqp,
                tc.tile_pool(name="psst", bufs=2, space="PSUM") as psstp,
            ):
                wq_t = wqkvp.tile([128, KC, J], bf16)
                wk_t = wqkvp.tile([128, KC, J], bf16)
                wv_t = wqkvp.tile([128, KC, J], bf16)

                # ----- phase 1: both parts with per-512-block x tiles.
                # The mem-part finalize (stats math + k/v epilogues, a
                # long serial DVE chain) is emitted inside the hid loop
                # so it overlaps the hid projection streams. -----
                ropep = tc.alloc_tile_pool(name="rope", bufs=1)
                cos_t = ropep.tile([128, S], bf16)
                sin_t = ropep.tile([128, S], bf16)
                with (
                    tc.tile_pool(name="xm", bufs=2) as xmp,
                    tc.tile_pool(name="sqa", bufs=8) as sqap,
                    tc.tile_pool(name="rowa", bufs=1) as rowap,
                ):
                    nc.sync.dma_start(wk_t[:], wk_e[:])
                    nc.scalar.dma_start(wv_t[:], wv_e[:])
                    nc.sync.dma_start(wq_t[:], wq_e[:])
                    row16a = rowap.tile([128, S], bf16, name="r16_mem",
                                        tag="r16m")
                    row16b = rowap.tile([128, S], bf16, name="r16_hid",
                                        tag="r16h")
                    for nb in range(NB):
                        x_nb = xmp.tile([128, KC, NBLK], bf16,
                                        name="x_nb")
                        for kc in range(KC):
                            eng = nc.sync if kc % 2 == 0 else nc.scalar
                            eng.dma_start(
                                x_nb[:, kc, :],
                                xm_e[:, kc, nb * NBLK:(nb + 1) * NBLK])
                        proj_raw_nb(wk_t, kT, 0, x_nb, nb, psqp)
                        # v_mem row-major: lhsT = xm l-tile, rhs = wv
                        for li in range(4):
                            lt = nb * 4 + li
                            ps = psqp.tile([128, J], f32, name="vm_ps",
                                           bufs=2)
                            for kc in range(KC):
                                nc.tensor.matmul(
                                    ps[:],
                                    x_nb[:, kc, li * 128:(li + 1) * 128],
                                    wv_t[:, kc, :],
                                    start=(kc == 0), stop=(kc == KC - 1))
                            nc.vector.tensor_copy(v_mem[:, lt, :], ps[:])
                        stats_nb(x_nb, nb, sqap, psstp, row16a)
                    for nb in range(NB):
                        x_nb = xmp.tile([128, KC, NBLK], bf16,
                                        name="x_nb")
                        for kc in range(KC):
                            eng = nc.sync if kc % 2 == 0 else nc.scalar
                            eng.dma_start(
                                x_nb[:, kc, :],
                                xh_e[:, kc, nb * NBLK:(nb + 1) * NBLK])
                        if nb == 0:
                            load_rope_tables()
                        proj_raw_nb(wq_t, qT, 0, x_nb, nb, psqp)
                        proj_raw_nb(wk_t, kT, S, x_nb, nb, psqp)
                        proj_raw_nb(wv_t, vTh, 0, x_nb, nb, psqp)
                        stats_nb(x_nb, nb, sqap, psstp, row16b)
                        # per-block hid finalize: epilogues + rope for
                        # block nb run while later blocks still project
                        rstd5, c5 = fin_nb(row16b, nb, sqap, psstp)
                        proj_epi_nb(qT, 0, nb, rstd5, c5, wsq_t, bq_t,
                                    sqap)
                        rope_packed_nb(
                            [(qT[0], 0, 0), (qT[0], 64, 0),
                             (qT[1], 0, 0), (qT[1], 64, 0),
                             (kT[0], 0, 0), (kT[0], 64, 0),
                             (kT[1], 0, 0), (kT[1], 64, 0)], "a", nb)
                        proj_epi_nb(kT, S, nb, rstd5, c5, wsk_t, bk_t,
                                    sqap)
                        rope_packed_nb(
                            [(kT[0], 0, S), (kT[0], 64, S),
                             (kT[1], 0, S), (kT[1], 64, S)], "b", nb)
                        proj_epi_nb(vTh, 0, nb, rstd5, c5, wsvc_t, bvc_t,
                                    sqap)
                        if nb == 0:
                            bcast_rows(wsvb, wsv_row, J, psqp, onesf)
                            bcast_rows(bvb, bv_row, J, psqp, onesf)
                        # per-block MEM finalize: k epilogue + v_mem
                        # epilogue for mem-block nb
                        rstd5m, c5m = fin_nb(row16a, nb, sqap, psstp)
                        proj_epi_nb(kT, 0, nb, rstd5m, c5m, wsk_t, bk_t,
                                    sqap)
                        # column-layout stats for the v_mem epilogue via
                        # a tiny DRAM round-trip on the gpsimd queue
                        for nm, row, col in (
                                ("rstd", rstd5m, rstd_col_mem),
                                ("c", c5m, c_col_mem)):
                            dr = dramp.tile([NBLK], bf16,
                                            name=f"st_{nm}_{nb}")
                            nc.gpsimd.dma_start(
                                dr[:].rearrange("(o a) -> o a", o=1),
                                row[0:1, :])
                            col4 = sqap.tile([128, 4], bf16,
                                             name=f"c4_{nm}",
                                             tag="col4", bufs=4)
                            nc.gpsimd.dma_start(
                                col4[:],
                                dr[:].rearrange("(t p) -> p t", p=128))
                            nc.vector.tensor_copy(
                                col[:, 4 * nb:4 * nb + 4], col4[:])
                        for li in range(4):
                            lt = nb * 4 + li
                            cwv = sqap.tile([128, J], f32,
                                            name="cwv", bufs=2)
                            nc.vector.tensor_scalar(
                                out=cwv[:], in0=wsvb[:],
                                scalar1=c_col_mem[:, lt:lt + 1],
                                scalar2=None, op0=Alu.mult)
                            nc.vector.scalar_tensor_tensor(
                                out=v_mem[:, lt, :],
                                in0=v_mem[:, lt, :],
                                scalar=rstd_col_mem[:, lt:lt + 1],
                                in1=cwv[:], op0=Alu.mult,
                                op1=Alu.subtract)
                            nc.vector.tensor_add(
                                v_mem[:, lt, :], v_mem[:, lt, :],
                                bvb[:])
                ropep.release()

            # ---------- loop 1: attention + local o_p + AllReduce ---------
            op_dram = [dramp.tile([D, NBLK], bf16, name=f"op_d{b}")
                       for b in range(NB)]
            h_sh = [dshp.tile([D, NBLK], bf16, name=f"h_sh{b}",
                              addr_space="Shared")
                    for b in range(NB)]
            with (
                tc.tile_pool(name="maskp", bufs=1) as maskp,
                tc.tile_pool(name="wop", bufs=1) as wop,
                tc.tile_pool(name="attw", bufs=8) as attwp,
                tc.tile_pool(name="attt", bufs=4) as atttp,
                tc.tile_pool(name="cmbp", bufs=3) as cmbp,
                tc.tile_pool(name="attr", bufs=1) as attrp,
                tc.tile_pool(name="oc", bufs=2) as ocp,
                tc.tile_pool(name="psS", bufs=4, space="PSUM") as psSp,
                tc.tile_pool(name="psA", bufs=1, space="PSUM") as psAp,
                tc.tile_pool(name="psD", bufs=1, space="PSUM") as psDp,
            ):
                masks_t = maskp.tile([128, 4, NBLK], bf16)
                nc.sync.dma_start(masks_t[:], masks_e[:])
                h0sq_t = maskp.tile([128, KC, NBLK], bf16)
                wo_t = wop.tile([128, MD, D], bf16)
                nc.sync.dma_start(wo_t[:], wo_e[:])
                for b in range(NB):
                    bsl = slice(b * NBLK, (b + 1) * NBLK)
                    T = 4 * b + 4
                    if b == 1:
                        # prefetch the gate first half behind o stores
                        nc.sync.dma_start(wg_h1[:],
                                          wg_e[:, :, :FFL // 2])
                    if b == 3:
                        # h0 = o_sum(0) + xh(0) and its squares, built on
                        # DMA + gpsimd only (Scalar is block 3's pacer)
                        for tq in range(4):
                            eng = nc.sync if tq % 2 == 0 else nc.scalar
                            eng.dma_start(
                                h0_t[:, tq * 4:(tq + 1) * 4, :],
                                h_sh[0][tq * 512:(tq + 1) * 512, :]
                                .rearrange("(t p) s -> p t s", p=128))
                        for kc in range(KC):
                            xh0 = atttp.tile([128, NBLK], bf16,
                                             name="xh0", tag="xh0",
                                             bufs=2)
                            eng = nc.sync if kc % 2 == 0 else nc.scalar
                            eng.dma_start(xh0[:], xh_e[:, kc, 0:NBLK])
                            nc.gpsimd.tensor_add(h0_t[:, kc, :],
                                                 h0_t[:, kc, :], xh0[:])
                            nc.gpsimd.tensor_mul(h0sq_t[:, kc, :],
                                                 h0_t[:, kc, :],
                                                 h0_t[:, kc, :])
                    den4 = psDp.tile([128, NBLK], f32, name="den4")
                    sf4 = psSp.tile([128, NBLK], f32, name="sf4",
                                    tag="sbrb", bufs=1)
                    dent4 = attrp.tile([128, NBLK], f32, name="dent4")
                    swb4 = attrp.tile([128, NBLK], bf16, name="swb4")
                    rcpb4 = attrp.tile([128, NBLK], bf16, name="rcpb4")
                    ap_ps = [psAp.tile([128, NBLK], f32, name=f"ap{m}",
                                       bufs=1)
                             for m in range(2)]
                    # self-key q.k products hoisted: they only need the
                    # rope'd q/k, so the DVE does them while the PE runs
                    # the score matmuls; one full-tile mul covers both
                    # heads of an m group
                    qks = []
                    for m in range(2):
                        qk = atttp.tile([128, NBLK], bf16,
                                        name=f"qk{m}",
                                        tag=f"qk{m}", bufs=2)
                        nc.vector.tensor_mul(
                            qk[:, :], qT[m][:, bsl],
                            kT[m][:, S + b * NBLK:S + (b + 1) * NBLK])
                        qks.append(qk)

                    def q_lo(t):
                        """First unmasked q column for key-tile t (the
                        diagonal band is strictly causal: keys t*128+ii
                        only reach q > t*128+ii within the block)."""
                        return max(0, (t - 4 * b) * 128)

                    def s_pair(t):
                        """Score MMs for key-tile t, all 4 heads: two
                        row-tiled concurrent pairs, band-narrowed."""
                        tsl = slice(t * 128, (t + 1) * 128)
                        c0 = q_lo(t)
                        ss = []
                        for m in range(2):
                            for o in (0, 64):
                                hsl = slice(o, o + 64)
                                s_ps = psSp.tile([128, NBLK], f32,
                                                 name="s_ps")
                                nc.tensor.matmul(
                                    s_ps[:, c0:], kT[m][hsl, tsl],
                                    qT[m][hsl,
                                          b * NBLK + c0:(b + 1) * NBLK],
                                    start=True, stop=True,
                                    tile_position=(o, 0))
                                ss.append(s_ps)
                        return ss

                    ss_cur = s_pair(0)
                    for t in range(T):
                        ss_next = s_pair(t + 1) if t + 1 < T else None
                        c0 = q_lo(t)
                        # exp (+ mask on the diagonal band) on ScalarE/DVE
                        ws = []
                        for i, (m, o) in enumerate(
                                ((0, 0), (0, 64), (1, 0), (1, 64))):
                            w_t = attwp.tile([128, NBLK], bf16, name="w_t")
                            nc.scalar.activation(
                                w_t[:, c0:], ss_cur[i][:, c0:], AF.Exp,
                                scale=0.125)
                            if t >= 4 * b:
                                nc.vector.tensor_mul(
                                    w_t[:, c0:], w_t[:, c0:],
                                    masks_t[:, t - 4 * b, c0:])
                            ws.append(w_t)
                        # AV: col-tiled concurrent pairs per m
                        for m in range(2):
                            for io, o in enumerate((0, 64)):
                                nc.tensor.matmul(
                                    ap_ps[m][o:o + 64, c0:],
                                    v_mem[:, t,
                                          m * 128 + o:m * 128 + o + 64],
                                    ws[2 * m + io][:, c0:],
                                    start=(t == 0), stop=(t == T - 1),
                                    tile_position=(0, o))
                        # denominators: concurrent pairs at cols r
                        for m in range(2):
                            for io, o in enumerate((0, 64)):
                                r = 32 * (2 * m + io)
                                nc.tensor.matmul(
                                    den4[r:r + 1, c0:], ones_c[:, 0:1],
                                    ws[2 * m + io][:, c0:],
                                    start=(t == 0), stop=(t == T - 1),
                                    tile_position=(0, r))
                        ss_cur = ss_next

                    # self key: sf matmuls per head, then the whole
                    # denominator chain as full-tile ops (the valid rows
                    # sit at partitions 0/32/64/96; the other partitions
                    # carry garbage that is never read back)
                    heads = [(m, io, o) for m in range(2)
                             for io, o in enumerate((0, 64))]
                    for m, io, o in heads:
                        hsl = slice(o, o + 64)
                        nc.tensor.matmul(
                            sf4[32 * (2 * m + io):32 * (2 * m + io) + 1,
                                :],
                            ones_c[hsl, 0:1], qks[m][hsl, :],
                            start=True, stop=True,
                            tile_position=(o, 32 * (2 * m + io)))
                    nc.scalar.activation(swb4[:], sf4[:], AF.Exp,
                                         scale=0.125)
                    nc.vector.tensor_add(dent4[:], den4[:], swb4[:])
                    lnd = attrp.tile([128, NBLK], f32, name="lnd")
                    nc.scalar.activation(lnd[:], dent4[:], AF.Ln)
                    nc.scalar.activation(rcpb4[:], lnd[:], AF.Exp,
                                         scale=-1.0)
                    # broadcast self_w and 1/den to each head's 64 rows
                    for m in range(2):
                        sb_ps = psSp.tile([128, NBLK], f32, name="sb_ps",
                                          tag="sbrb", bufs=1)
                        rb_ps = psSp.tile([128, NBLK], f32, name="rb_ps",
                                          tag="sbrb", bufs=1)
                        for io, o in enumerate((0, 64)):
                            r = 32 * (2 * m + io)
                            rsl = slice(r, r + 1)
                            nc.tensor.matmul(
                                sb_ps[o:o + 64, :], ones128[rsl, 0:64],
                                swb4[rsl, :], start=True, stop=True,
                                tile_position=(r, o))
                            nc.tensor.matmul(
                                rb_ps[o:o + 64, :], ones128[rsl, 0:64],
                                rcpb4[rsl, :], start=True, stop=True,
                                tile_position=(r, o))
                        # combine: (attn + self_w * vTh) / den
                        t0 = atttp.tile([128, NBLK], bf16, name="t0")
                        nc.vector.tensor_mul(t0[:], vTh[m][:, bsl],
                                             sb_ps[:])
                        t1 = atttp.tile([128, NBLK], bf16, name="t1")
                        nc.vector.tensor_add(t1[:], ap_ps[m][:], t0[:])
                        cmb = cmbp.tile([128, NBLK], bf16, name=f"cmb{m}")
                        nc.vector.tensor_mul(cmb[:], t1[:], rb_ps[:])
                        if m == 0:
                            cmb0 = cmb
                        else:
                            cmb1 = cmb

                    # local o_p: full-D partial from this core's heads
                    # (attention only; the residual joins in the down-proj
                    # partial instead), stored in 512-row quarters
                    for q in range(4):
                        oc_q = ocp.tile([128, 4, NBLK], bf16,
                                        name="oc_q", tag="oc_q", bufs=2)
                        for sub in range(4):
                            md16 = q * 4 + sub
                            ps = psSp.tile([128, NBLK], f32, name="o_ps",
                                           tag="s_ps")
                            nc.tensor.matmul(
                                ps[:],
                                wo_t[:, 0, md16 * 128:(md16 + 1) * 128],
                                cmb0[:], start=True, stop=False)
                            nc.tensor.matmul(
                                ps[:],
                                wo_t[:, 1, md16 * 128:(md16 + 1) * 128],
                                cmb1[:], start=False, stop=True)
                            if sub % 2 == 0:
                                nc.vector.tensor_copy(
                                    oc_q[:, sub, :], ps[:])
                            else:
                                nc.scalar.copy(oc_q[:, sub, :], ps[:])
                        eng = nc.sync if q % 2 == 0 else nc.scalar
                        eng.dma_start(
                            op_dram[b][q * 512:(q + 1) * 512, :].rearrange(
                                "(t p) s -> p t s", p=128),
                            oc_q[:])
                    # one AllReduce per block: every core gets the full
                    # o-sum for this block's columns
                    nc.gpsimd.collective_compute(
                        "AllReduce", mybir.AluOpType.add,
                        replica_groups=rg,
                        ins=[op_dram[b].opt()], outs=[h_sh[b].opt()])
                    if b == 3:
                        # LN2 stats for block 0 (PE/DVE/2 Scalar ops run
                        # in block 3's engine slack / the AR3 window)
                        st0 = psSp.tile([128, NBLK], f32, name="st0",
                                        tag="sbrb", bufs=1)
                        for kc in range(KC):
                            nc.tensor.matmul(
                                st0[0:1, :], ones_c[:], h0_t[:, kc, :],
                                start=(kc == 0), stop=(kc == KC - 1),
                                tile_position=(0, 0))
                            nc.tensor.matmul(
                                st0[32:33, :], ones_c[:],
                                h0sq_t[:, kc, :],
                                start=(kc == 0), stop=(kc == KC - 1),
                                tile_position=(0, 32))
                        r160 = atttp.tile([128, NBLK], bf16,
                                          name="r160", tag="r160",
                                          bufs=1)
                        nc.vector.tensor_copy(r160[0:1, :], st0[0:1, :])
                        nc.vector.tensor_copy(r160[32:33, :],
                                              st0[32:33, :])
                        ps0 = psSp.tile([128, NBLK], f32, name="bc0",
                                        tag="sbrb", bufs=1)
                        nc.tensor.matmul(ps0[:], invD128[0:1, :],
                                         r160[0:1, :],
                                         start=True, stop=True)
                        mean0 = atttp.tile([128, NBLK], bf16,
                                           name="mean0", tag="mean0",
                                           bufs=1)
                        nc.vector.tensor_copy(mean0[:], ps0[:])
                        ps0b = psSp.tile([128, NBLK], f32, name="bc0b",
                                         tag="sbrb", bufs=1)
                        nc.tensor.matmul(ps0b[:], invD128[32:33, :],
                                         r160[32:33, :],
                                         start=True, stop=True)
                        ex20 = atttp.tile([128, NBLK], bf16,
                                          name="ex20", tag="ex20",
                                          bufs=1)
                        nc.vector.tensor_copy(ex20[:], ps0b[:])
                        m20 = atttp.tile([128, NBLK], bf16, name="m20",
                                         tag="m20", bufs=1)
                        nc.vector.tensor_mul(m20[:], mean0[:], mean0[:])
                        nc.vector.tensor_sub(ex20[:], ex20[:], m20[:])
                        nc.scalar.activation(ex20[:], ex20[:], AF.Ln,
                                             bias=eps_c[:])
                        nc.scalar.activation(rstd05[:], ex20[:], AF.Exp,
                                             scale=-0.5)
                        nc.vector.tensor_mul(c05[:], mean0[:],
                                             rstd05[:])
            kqvp.release()

            # second halves of gate/up + the down weight load into the
            # space the attention pools and kqv freed
            wudp = tc.alloc_tile_pool(name="wud", bufs=1)
            wu_h1 = wudp.tile([128, KC, FFL // 2], bf16)
            wg_h2 = wudp.tile([128, KC, FFL // 2], bf16)
            wu_h2 = wudp.tile([128, KC, FFL // 2], bf16)
            wd_t = wudp.tile([128, FFL // 128, D], bf16)

            # ---------- loop 2: LN2 + gated MLP + down + out --------------
            # row-parallel down: each core contracts its own FFL slice of
            # m into a full-D partial which also carries ident8 @ h (the
            # residual + o_sum, scaled 1/8); the per-block ReduceScatter
            # then hands back this core's own rows of h + down_sum — the
            # final output rows, with no separate residual path.
            dp_dram = [dramp.tile([D, NBLK], bf16, name=f"dp_d{b}")
                       for b in range(NB - 1)]
            dp_rs = [dramp.tile([J, NBLK], bf16, name=f"dp_rs{b}")
                     for b in range(NB - 1)]
            dp3_dram = [dramp.tile([D, NBLK // 4], bf16, name=f"dp3_d{i}")
                        for i in range(4)]
            dp3_rs = [dramp.tile([J, NBLK // 4], bf16, name=f"dp3_rs{i}")
                      for i in range(4)]
            with (
                tc.tile_pool(name="hblk", bufs=2) as hblkp,
                tc.tile_pool(name="xh2", bufs=2) as xh2p,
                tc.tile_pool(name="sq5", bufs=2) as sq5p,
                tc.tile_pool(name="sm5", bufs=1) as sm5p,
                tc.tile_pool(name="mloc", bufs=1) as mlocp,
                tc.tile_pool(name="gut", bufs=2) as gutp,
                tc.tile_pool(name="dcp", bufs=2) as dcp,
                tc.tile_pool(name="outt", bufs=1) as outtp,
                tc.tile_pool(name="psG", bufs=2, space="PSUM") as psGp,
                tc.tile_pool(name="psU", bufs=2, space="PSUM") as psUp,
                tc.tile_pool(name="psst5", bufs=1, space="PSUM") as psst5p,
                tc.tile_pool(name="psDn", bufs=2, space="PSUM") as psDnp,
            ):
                def h_load(b):
                    """h_t = o_sum (AllReduced) + xh, raw pre-LN2."""
                    bsl = slice(b * NBLK, (b + 1) * NBLK)
                    h_t = hblkp.tile([128, KC, NBLK], bf16, name="h_t")
                    for tq in range(4):
                        eng = nc.sync if tq % 2 == 0 else nc.scalar
                        eng.dma_start(
                            h_t[:, tq * 4:(tq + 1) * 4, :],
                            h_sh[b][tq * 512:(tq + 1) * 512, :].rearrange(
                                "(t p) s -> p t s", p=128))
                    for kc in range(KC):
                        xh2 = xh2p.tile([128, NBLK], bf16, name="xh2")
                        eng = nc.sync if kc % 2 == 0 else nc.scalar
                        eng.dma_start(xh2[:], xh_e[:, kc, bsl])
                        # adds on the (otherwise idle) gpsimd engine so
                        # the h chain never queues behind DVE work
                        nc.gpsimd.tensor_add(h_t[:, kc, :],
                                             h_t[:, kc, :], xh2[:])
                    return h_t

                def down_block(h_t, m_loc, clo, chi, dpd, dpr):
                    """Down partial over columns [clo, chi) + ident8 @ h;
                    ReduceScatter returns own rows of h + down_sum."""
                    w = chi - clo
                    for q in range(4):
                        dcq = dcp.tile([128, 4, NBLK], bf16, name="dcq")
                        for sub in range(4):
                            md16 = q * 4 + sub
                            ps = psDnp.tile([128, NBLK], f32, name="d_ps")
                            nc.tensor.matmul(
                                ps[:, :w], ident8[:],
                                h_t[:, md16, clo:chi],
                                start=True, stop=False)
                            for fc in range(FFL // 128):
                                nc.tensor.matmul(
                                    ps[:, :w],
                                    wd_t[:, fc,
                                         md16 * 128:(md16 + 1) * 128],
                                    m_loc[:, fc, clo:chi],
                                    start=False,
                                    stop=(fc == FFL // 128 - 1))
                            nc.scalar.copy(dcq[:, sub, :w], ps[:, :w])
                        eng = nc.sync if q % 2 == 0 else nc.scalar
                        eng.dma_start(
                            dpd[q * 512:(q + 1) * 512, :].rearrange(
                                "(t p) s -> p t s", p=128),
                            dcq[:, :, :w])
                    nc.gpsimd.collective_compute(
                        "ReduceScatter", mybir.AluOpType.add,
                        replica_groups=rg,
                        ins=[dpd.opt()], outs=[dpr.opt()])

                def ln2_block(b, h_t):
                    """LN2 stats on raw h; returns broadcast (rstd, c)
                    tiles. gu runs on RAW h with the LN correction folded
                    into its epilogue, so the normalize never sits on the
                    inter-block critical path."""
                    stp = psst5p.tile([128, NBLK], f32, name="st5")
                    for kc in range(KC):
                        sq_t = sq5p.tile([128, NBLK], bf16, name="sq5_t",
                                          bufs=8)
                        nc.scalar.activation(sq_t[:], h_t[:, kc, :],
                                             AF.Square)
                        nc.tensor.matmul(
                            stp[0:1, :], ones_c[:], h_t[:, kc, :],
                            start=(kc == 0), stop=(kc == KC - 1),
                            tile_position=(0, 0))
                        nc.tensor.matmul(
                            stp[32:33, :], ones_c[:], sq_t[:],
                            start=(kc == 0), stop=(kc == KC - 1),
                            tile_position=(0, 32))
                    r16 = sm5p.tile([128, NBLK], bf16, name="r165",
                                    tag="sm5r", bufs=1)
                    nc.vector.tensor_copy(r16[0:1, :], stp[0:1, :])
                    nc.vector.tensor_copy(r16[32:33, :], stp[32:33, :])
                    mean_b = sm5p.tile([128, NBLK], bf16, name="mean5b",
                                       tag="sm5m", bufs=1)
                    ex2_b = sm5p.tile([128, NBLK], bf16, name="ex25b",
                                      tag="sm5e", bufs=1)
                    ps = psst5p.tile([128, NBLK], f32, name="bc_ps",
                                     tag="bc_ps", bufs=1)
                    nc.tensor.matmul(ps[:], invD128[0:1, :], r16[0:1, :],
                                     start=True, stop=True)
                    nc.vector.tensor_copy(mean_b[:], ps[:])
                    ps2 = psst5p.tile([128, NBLK], f32, name="bc_ps2",
                                      tag="bc_ps", bufs=1)
                    nc.tensor.matmul(ps2[:], invD128[32:33, :],
                                     r16[32:33, :],
                                     start=True, stop=True)
                    nc.vector.tensor_copy(ex2_b[:], ps2[:])
                    m2_b = sm5p.tile([128, NBLK], bf16, name="m25b",
                                     tag="sm5m2", bufs=1)
                    nc.vector.tensor_mul(m2_b[:], mean_b[:], mean_b[:])
                    nc.vector.tensor_sub(ex2_b[:], ex2_b[:], m2_b[:])
                    rstd_b = sm5p.tile([128, NBLK], bf16, name="rstd5b",
                                       tag="sm5rs", bufs=2)
                    lnv5 = sm5p.tile([128, NBLK], f32, name="lnv5",
                                     tag="sm5ln", bufs=1)
                    nc.scalar.activation(lnv5[:], ex2_b[:], AF.Ln,
                                         bias=eps_c[:])
                    nc.scalar.activation(rstd_b[:], lnv5[:], AF.Exp,
                                         scale=-0.5)
                    c_bb = sm5p.tile([128, NBLK], bf16, name="c5b",
                                     tag="sm5c", bufs=2)
                    nc.vector.tensor_mul(c_bb[:], mean_b[:], rstd_b[:])
                    return rstd_b, c_bb

                def gu_block(b, h_t, rstd_b, c_bb):
                    """Gate/up on RAW h; LN2 folded into the epilogue:
                    g' = psg*rstd - (c*wsg - bg), same for u."""
                    m_loc = mlocp.tile([128, FFL // 128, NBLK], bf16,
                                       name="m_loc")
                    for mf in range(FFL // 128):
                        if mf < FFL // 256:
                            wgs, wus, mfl = wg_h1, wu_h1, mf
                        else:
                            wgs, wus, mfl = wg_h2, wu_h2, mf - FFL // 256
                        psg = psGp.tile([128, NBLK], f32, name="g_ps")
                        psu = psUp.tile([128, NBLK], f32, name="u_ps")
                        for kc in range(KC):
                            nc.tensor.matmul(
                                psg[:],
                                wgs[:, kc, mfl * 128:(mfl + 1) * 128],
                                h_t[:, kc, :],
                                start=(kc == 0), stop=(kc == KC - 1))
                            nc.tensor.matmul(
                                psu[:],
                                wus[:, kc, mfl * 128:(mfl + 1) * 128],
                                h_t[:, kc, :],
                                start=(kc == 0), stop=(kc == KC - 1))
                        cwg = gutp.tile([128, NBLK], bf16, name="cwg")
                        nc.vector.tensor_scalar(
                            out=cwg[:], in0=c_bb[:],
                            scalar1=wsg_t[:, mf:mf + 1],
                            scalar2=bg_t[:, mf:mf + 1],
                            op0=Alu.mult, op1=Alu.subtract)
                        g_t = gutp.tile([128, NBLK], bf16, name="g_t")
                        nc.vector.tensor_mul(g_t[:], psg[:], rstd_b[:])
                        nc.vector.tensor_sub(g_t[:], g_t[:], cwg[:])
                        cwu = gutp.tile([128, NBLK], bf16, name="cwu")
                        nc.vector.tensor_scalar(
                            out=cwu[:], in0=c_bb[:],
                            scalar1=wsu_t[:, mf:mf + 1],
                            scalar2=bu_t[:, mf:mf + 1],
                            op0=Alu.mult, op1=Alu.subtract)
                        u_t = gutp.tile([128, NBLK], bf16, name="u_t")
                        nc.vector.tensor_mul(u_t[:], psu[:], rstd_b[:])
                        nc.vector.tensor_sub(u_t[:], u_t[:], cwu[:])
                        sg = gutp.tile([128, NBLK], bf16, name="sg")
                        nc.scalar.activation(sg[:], g_t[:], AF.Sigmoid)
                        silu = gutp.tile([128, NBLK], bf16, name="silu")
                        nc.vector.tensor_mul(silu[:], g_t[:], sg[:])
                        nc.vector.tensor_mul(m_loc[:, mf, :], silu[:],
                                             u_t[:])
                    return m_loc

                def out_block(b, clo, chi, dpr):
                    """Own jsl rows of h + down_sum -> out (f32).

                    The rsd load waits on the ReduceScatter, so it
                    rides the gpsimd queue: on sync/scalar it would
                    head-of-line block the next block's h loads and
                    Square stream behind that wait."""
                    w = chi - clo
                    rsd = outtp.tile([128, MD, NBLK], bf16, name="rsd")
                    nc.gpsimd.dma_start(
                        rsd[:, :, :w], dpr[:, :].rearrange(
                            "(t p) s -> p t s", p=128))
                    for md in range(MD):
                        ot = outtp.tile([128, NBLK], f32, name="ot")
                        nc.vector.tensor_copy(ot[:, :w], rsd[:, md, :w])
                        nc.gpsimd.dma_start(
                            out_e[md, :, b * NBLK + clo:b * NBLK + chi],
                            ot[:, :w])

                # per-block pipeline; the final block's down/RS/out is
                # split into column halves to shorten the exposed tail
                h_cur = h0_t
                st_cur = (rstd05, c05)
                # weights: wu first half first (gu's first up-matmuls
                # need it almost immediately)
                nc.scalar.dma_start(wu_h1[:], wu_e[:, :, :FFL // 2])
                nc.sync.dma_start(wg_h2[:], wg_e[:, :, FFL // 2:])
                nc.scalar.dma_start(wu_h2[:], wu_e[:, :, FFL // 2:])
                nc.sync.dma_start(wd_t[:, :FFL // 256, :],
                                  wd_e[:, :FFL // 256, :])
                nc.scalar.dma_start(wd_t[:, FFL // 256:, :],
                                    wd_e[:, FFL // 256:, :])
                # ln2(b+1)'s serial Square/stats chain is emitted before
                # gu(b)/down(b) so it overlaps their PE streams
                for b in range(NB):
                    if b + 1 < NB:
                        h_next = h_load(b + 1)
                        st_next = ln2_block(b + 1, h_next)
                    else:
                        h_next = st_next = None
                    ml = gu_block(b, h_cur, *st_cur)
                    if b < NB - 1:
                        down_block(h_cur, ml, 0, NBLK,
                                   dp_dram[b], dp_rs[b])
                        out_block(b, 0, NBLK, dp_rs[b])
                    else:
                        for qq in range(4):
                            qlo = qq * (NBLK // 4)
                            qhi = (qq + 1) * (NBLK // 4)
                            down_block(h_cur, ml, qlo, qhi,
                                       dp3_dram[qq], dp3_rs[qq])
                            out_block(b, qlo, qhi, dp3_rs[qq])
                    h_cur, st_cur = h_next, st_next
            wudp.release()
            h0p.release()
            wguh1p.release()

    return nc


# ---------------------------------------------------------------------------
# Host side
# ---------------------------------------------------------------------------

def _chunkT(a):
    """[R, D] -> [128, D//128, R] view for lhsT/rhs chunk layout.

    Result[p, kc, r] = a[r, kc*128 + p].
    """
    R, Dd = a.shape
    return np.ascontiguousarray(
        a.reshape(R, Dd // 128, 128).transpose(2, 1, 0))


def prepare_inputs(hidden_states, memory, position_ids,
                   ln1_w, ln1_b, ln2_w, ln2_b,
                   Wq, Wk, Wv, Wo, Wg, Wu, Wd, S):
    """Build the 8 per-core in_maps (numpy host prep)."""
    f32 = np.float32
    hid = np.asarray(hidden_states, f32)[0]       # [S, D]
    mem = np.asarray(memory, f32)[0]
    pos = np.asarray(position_ids)[0].astype(np.float64)

    Wq1 = np.asarray(Wq, f32) * np.asarray(ln1_w, f32)[None, :]
    Wk1 = np.asarray(Wk, f32) * np.asarray(ln1_w, f32)[None, :]
    Wv1 = np.asarray(Wv, f32) * np.asarray(ln1_w, f32)[None, :]
    bq = np.asarray(Wq, f32) @ np.asarray(ln1_b, f32)
    bk = np.asarray(Wk, f32) @ np.asarray(ln1_b, f32)
    bv = np.asarray(Wv, f32) @ np.asarray(ln1_b, f32)
    Wg2 = np.asarray(Wg, f32) * np.asarray(ln2_w, f32)[None, :]
    Wu2 = np.asarray(Wu, f32) * np.asarray(ln2_w, f32)[None, :]
    bg = np.asarray(Wg, f32) @ np.asarray(ln2_b, f32)
    bu = np.asarray(Wu, f32) @ np.asarray(ln2_b, f32)
    Wo_ = np.asarray(Wo, f32)
    Wd_ = np.asarray(Wd, f32)

    # x^T chunk layouts (shared by all cores)
    xm = _chunkT(mem).astype(BF16)                # [128, KC, S]
    xh = _chunkT(hid).astype(BF16)

    # rope tables [128, 2S], row pattern period 16
    inv = BASE ** (-(np.arange(8, dtype=np.float64) * 2) / RD)
    t = pos[:, None] * inv[None, :]               # [S, 8]
    cos8 = np.cos(t).T                            # [8, S]
    sin8 = np.sin(t).T
    cos16 = np.concatenate([cos8, cos8], 0)       # [16, S]
    sin16 = np.concatenate([-sin8, sin8], 0)
    cosf = np.tile(np.concatenate([cos16, cos16], 1), (8, 1))  # [128, 2S]
    sinf = np.tile(np.concatenate([sin16, sin16], 1), (8, 1))
    cosf = cosf.astype(BF16)
    sinf = sinf.astype(BF16)

    ident8 = (np.eye(128) * 0.125).astype(BF16)

    # strict-causal masks for the 4 diagonal-band offsets
    ii = np.arange(128)[:, None]
    jj = np.arange(NBLK)[None, :]
    masks = np.stack(
        [(ii + 128 * o < jj) for o in range(4)], 1).astype(BF16)  # [128,4,512]

    in_maps = []
    for c in range(N_CORES):
        jsl = slice(c * J, (c + 1) * J)
        fsl = slice(c * FFL, (c + 1) * FFL)
        wq_c = Wq1[jsl]                            # [J, D]
        wk_c = Wk1[jsl]
        wv_c = Wv1[jsl]
        im = {
            "xm": xm, "xh": xh,
            "ident8": ident8,
            "wq": _chunkT(wq_c).astype(BF16),
            "wk": _chunkT(wk_c).astype(BF16),
            "wv": _chunkT(wv_c).astype(BF16),
            "wo_p": _chunkT(Wo_[:, jsl]).astype(BF16),
            "wg": _chunkT(Wg2[fsl]).astype(BF16),
            "wu": _chunkT(Wu2[fsl]).astype(BF16),
            "wd": _chunkT(Wd_[:, fsl]).astype(BF16),
            "wsq": np.ascontiguousarray(
                wq_c.sum(1).reshape(MD, 128).T).astype(f32),
            "wsk": np.ascontiguousarray(
                wk_c.sum(1).reshape(MD, 128).T).astype(f32),
            "wsvc": np.ascontiguousarray(
                wv_c.sum(1).reshape(MD, 128).T).astype(f32),
            "bq": np.ascontiguousarray(
                bq[jsl].reshape(MD, 128).T).astype(f32),
            "bk": np.ascontiguousarray(
                bk[jsl].reshape(MD, 128).T).astype(f32),
            "bvc": np.ascontiguousarray(
                bv[jsl].reshape(MD, 128).T).astype(f32),
            "wsv_row": wv_c.sum(1)[None, :].astype(f32),
            "bv_row": bv[jsl][None, :].astype(f32),
            "bg": np.ascontiguousarray(
                bg[fsl].reshape(FFL // 128, 128).T).astype(f32),
            "bu": np.ascontiguousarray(
                bu[fsl].reshape(FFL // 128, 128).T).astype(f32),
            "wsg": np.ascontiguousarray(
                Wg2[fsl].sum(1).reshape(FFL // 128, 128).T).astype(f32),
            "wsu": np.ascontiguousarray(
                Wu2[fsl].sum(1).reshape(FFL // 128, 128).T).astype(f32),
            "rope_cos": cosf, "rope_sinsg": sinf,
            "masks": masks,
        }
        in_maps.append(im)
    return in_maps


def assemble_output(results, S):
    outT = np.concatenate(
        [np.asarray(results[c]["out"]).reshape(J, S)
         for c in range(N_CORES)], 0)              # [D, S]
    return np.ascontiguousarray(outT.T).reshape(1, S, D).astype(np.float32)


_GRAPH_CACHE = {}


def get_graph(S):
    if S not in _GRAPH_CACHE:
        _GRAPH_CACHE[S] = build_graph(S)
    return _GRAPH_CACHE[S]


def kernel(hidden_states, memory, attention_mask, position_ids,
           ln1_w, ln1_b, ln2_w, ln2_b, Wq, Wk, Wv, Wo, Wg, Wu, Wd):
    from concourse.bass_utils import run_bass_kernel_spmd

    S = np.asarray(hidden_states).shape[1]
    in_maps = prepare_inputs(
        hidden_states, memory, position_ids, ln1_w, ln1_b, ln2_w, ln2_b,
        Wq, Wk, Wv, Wo, Wg, Wu, Wd, S)
    nc = get_graph(S)
    res = run_bass_kernel_spmd(nc, in_maps, core_ids=list(range(N_CORES)))
    return assemble_output(res.results, S)



# revision 38
# speedup vs baseline: 1.0241x; 1.0056x over previous
"""Trainium2 Bass kernel for nn_ArcDecoderLayer (sparse_attention).

Self-contained: takes FULL unsharded inputs, shards across 8 NeuronCores
(head-parallel attention, row-parallel o_proj with AllReduce, FF-parallel
MLP with AllGather of the intermediate), returns the FULL output.

v2 layout/schedule:
- LN1/LN2 statistics as concurrent col-tiled (sum, sumsq) matmul pairs;
  raw sums broadcast with 1/D folded into the broadcast constant so all
  stats math runs partition-aligned on full tiles.
- Attention: per-key-tile waves; score MMs row-tiled concurrent pairs,
  AV MMs col-tiled concurrent pairs, denominator MMs concurrent pairs,
  software-pipelined one key-tile ahead of the ScalarE exp; diagonal
  band tiles narrowed to their unmasked column range; packed full-width
  RoPE.
- o_proj computed locally from this core's heads into a full-D partial;
  per-block ReduceScatter(+AllGather) gives the residual rows and the
  full h sum (no attn/o AllGathers).
- MLP: gate/up per 512-block; row-parallel down-proj straight from the
  SBUF-resident m slice into a full-D partial, ReduceScattered back (no
  m AllGather, no 33MB of gathered-m reads).
All matmul compute bf16 with f32 PSUM accumulation.
"""

import sys
import types

sys.path.insert(0, "/opt/trn_rl_repo")

# ---- shim antenv.axon_hooks so trace=True profiling works in this image ----
if "antenv.axon_hooks" not in sys.modules:
    _hook_mod = types.ModuleType("antenv.axon_hooks")
    _hook_state = {"hook": None}

    def _set_hook(h):
        _hook_state["hook"] = h

    def _get_hook():
        return _hook_state["hook"]

    _hook_mod.set_axon_ntff_profile_hook = _set_hook
    _hook_mod.get_axon_ntff_profile_hook = _get_hook
    sys.modules["antenv.axon_hooks"] = _hook_mod
    try:
        import antenv

        antenv.axon_hooks = _hook_mod
        from trn_agent_boot.trn_boot import _ntff_profile_via_ctypes

        _set_hook(_ntff_profile_via_ctypes("/opt/axon/libaxon_pjrt.so"))
    except Exception:
        pass

import numpy as np
import ml_dtypes

import concourse.bass as bass
import concourse.mybir as mybir
import concourse.tile as tile
from concourse import library_config
from concourse.vector_clock import ScopedClock

BF16 = ml_dtypes.bfloat16

N_CORES = 8
D = 2048
FF = 8192
H = 32
DH = 64
RD = 16
EPS = 1e-5
BASE = 10000.0

J = D // N_CORES        # 256 head-dims per core (4 heads)
FFL = FF // N_CORES     # 1024 ff dims per core
KC = D // 128           # 16 contraction chunks
NBLK = 512              # lq block width
MD = J // 128           # 2 output Mtiles per core for down/out


WAIT_LIMITS = {"InstNoOp": 1, "InstDrain": 1, "InstEventSemaphore": 1}
DEFAULT_WAIT_LIMIT = 1


class PatchedTC(tile.TileContext):
    """TileContext patched for this walrus build, which rejects instructions
    carrying more than a couple of sync wait commands: excess waits are
    split onto injected same-engine nops just before the instruction."""

    _wsplit_n = 0

    def _split_excess_waits(self, ordered):
        for bb, insts in ordered.items():
            out = []
            for inst in insts:
                si = inst.sync_info
                waits = list(si.on_wait) if si and si.on_wait else []
                lim = WAIT_LIMITS.get(type(inst).__name__,
                                      DEFAULT_WAIT_LIMIT)
                if len(waits) > lim:
                    for w in waits[:-lim]:
                        nop = mybir.InstNoOp(
                            name=f"I-wsplit-{PatchedTC._wsplit_n}",
                            ins=[], outs=[], engine=inst.engine,
                            nofuse=True)
                        PatchedTC._wsplit_n += 1
                        nop.sync_info = mybir.SyncInfo(
                            on_wait=[w], on_update=[])
                        out.append(nop)
                    inst.sync_info = mybir.SyncInfo(
                        on_wait=waits[-lim:],
                        on_update=list(si.on_update or []))
                out.append(inst)
            ordered[bb] = out

    def _lower_ordered_insts(self, ordered):
        self._split_excess_waits(ordered)
        return super()._lower_ordered_insts(ordered)

    def _drain_and_barrier(self, tick_clock, wait_clock):
        nc = self.nc
        probe = nc.sync.nop(nofuse=True, hint="tail_wait_probe")
        wait_clock.add_sem_waits(
            probe.ins, ScopedClock({None: tick_clock.global_clock})
        )
        waits = list(probe.ins.sync_info.on_wait or [])
        probe.ins.sync_info.on_wait = waits[:1]
        for i in range(1, len(waits)):
            n = nc.sync.nop(nofuse=True, hint=f"tail_wait_{i}")
            n.ins.sync_info = mybir.SyncInfo(on_wait=[waits[i]], on_update=[])
        nc.sync.drain()
        nc.all_engine_barrier()
        assert self.sems is not None
        popped = nc._tile_sem_poison_stack.pop()
        assert popped is self._sem_poison
        nc.clear_and_free_semaphores(list(self.sems.allocated().values()))
        nc.all_engine_barrier()


def build_graph(S):
    """Build the SPMD 8-core graph for sequence length S (multiple of 512)."""
    dt = mybir.dt
    f32, bf16 = dt.float32, dt.bfloat16
    AF = mybir.ActivationFunctionType
    Alu = mybir.AluOpType
    NB = S // NBLK          # lq blocks (4)
    LT = S // 128           # 128-wide l tiles per part
    S2 = 2 * S

    nc = bass.Bass()
    P = nc.declare_dram_parameter

    xm_e = P("xm", [128, KC, S], bf16, isOutput=False)
    xh_e = P("xh", [128, KC, S], bf16, isOutput=False)
    ident8_e = P("ident8", [128, 128], bf16, isOutput=False)
    wq_e = P("wq", [128, KC, J], bf16, isOutput=False)
    wk_e = P("wk", [128, KC, J], bf16, isOutput=False)
    wv_e = P("wv", [128, KC, J], bf16, isOutput=False)
    wo_e = P("wo_p", [128, MD, D], bf16, isOutput=False)
    wg_e = P("wg", [128, KC, FFL], bf16, isOutput=False)
    wu_e = P("wu", [128, KC, FFL], bf16, isOutput=False)
    wd_e = P("wd", [128, FFL // 128, D], bf16, isOutput=False)
    # column (per-partition) weight rowsums + biases for q/k/vTh epilogues
    wsq_e = P("wsq", [128, 2], f32, isOutput=False)
    wsk_e = P("wsk", [128, 2], f32, isOutput=False)
    wsvc_e = P("wsvc", [128, 2], f32, isOutput=False)   # for vT_h epilogue
    bq_e = P("bq", [128, 2], f32, isOutput=False)
    bk_e = P("bk", [128, 2], f32, isOutput=False)
    bvc_e = P("bvc", [128, 2], f32, isOutput=False)
    # row layouts for v_mem epilogue
    wsv_e = P("wsv_row", [1, J], f32, isOutput=False)
    bv_e = P("bv_row", [1, J], f32, isOutput=False)
    bg_e = P("bg", [128, FFL // 128], f32, isOutput=False)
    bu_e = P("bu", [128, FFL // 128], f32, isOutput=False)
    wsg_e = P("wsg", [128, FFL // 128], f32, isOutput=False)
    wsu_e = P("wsu", [128, FFL // 128], f32, isOutput=False)
    ropec_e = P("rope_cos", [128, S2], bf16, isOutput=False)
    ropes_e = P("rope_sinsg", [128, S2], bf16, isOutput=False)
    masks_e = P("masks", [128, 4, NBLK], bf16, isOutput=False)
    out_e = P("out", [MD, 128, S], f32, isOutput=True)

    rg = [list(range(N_CORES))]

    with PatchedTC(nc) as tc:
        with (
            tc.tile_pool(name="const", bufs=1) as constp,
            tc.tile_pool(name="dram", bufs=1, space="DRAM") as dramp,
            tc.tile_pool(name="dsh", bufs=1, space="DRAM") as dshp,
        ):
            # first half of the gate weight lives below kqvp on the
            # pool stack so it can prefetch during attention and survive
            # into the MLP loop (releases stay LIFO)
            wguh1p = tc.alloc_tile_pool(name="wguh1", bufs=1)
            wg_h1 = wguh1p.tile([128, KC, FFL // 2], bf16)
            wu_h1 = wguh1p.tile([128, KC, FFL // 2], bf16)
            # block-0 h + LN2 stats, precomputed inside attention so the
            # MLP can start the moment attention drains
            h0p = tc.alloc_tile_pool(name="h0", bufs=1)
            h0_t = h0p.tile([128, KC, NBLK], bf16)
            rstd05 = h0p.tile([128, NBLK], bf16)
            c05 = h0p.tile([128, NBLK], bf16)
            kqvp = tc.alloc_tile_pool(name="kqv", bufs=1)
            ones_c = constp.tile([128, 1], bf16)
            nc.vector.memset(ones_c[:], 1.0)
            ones128 = constp.tile([128, 128], bf16)
            nc.vector.memset(ones128[:], 1.0)
            invD128 = constp.tile([128, 128], bf16)
            nc.vector.memset(invD128[:], 1.0 / D)
            eps_c = constp.tile([128, 1], f32)
            nc.vector.memset(eps_c[:], EPS)
            onesf = constp.tile([1, 128], f32)
            nc.vector.memset(onesf[:], 1.0)
            ident8 = constp.tile([128, 128], bf16)
            nc.gpsimd.dma_start(ident8[:], ident8_e[:])

            def bcast_rows(dst, src_row, width, pspool, ones_row):
                """dst[0:128, :width] = src_row[0, :width] via K=1 matmuls
                (partition_broadcast is not encodable by this walrus)."""
                for i in range(0, width, NBLK):
                    w = min(NBLK, width - i)
                    ps = pspool.tile([128, NBLK], f32, name="bc_ps",
                                     tag="bc_ps", bufs=1)
                    nc.tensor.matmul(ps[:, :w], ones_row[0:1, :],
                                     src_row[0:1, i:i + w],
                                     start=True, stop=True)
                    nc.vector.tensor_copy(dst[:, i:i + w], ps[:, :w])

            wsvb = constp.tile([128, J], f32)
            wsv_row = constp.tile([1, J], f32)
            nc.sync.dma_start(wsv_row[:], wsv_e[:])
            bvb = constp.tile([128, J], f32)
            bv_row = constp.tile([1, J], f32)
            nc.sync.dma_start(bv_row[:], bv_e[:])
            wsq_t = constp.tile([128, 2], f32)
            nc.sync.dma_start(wsq_t[:], wsq_e[:])
            wsk_t = constp.tile([128, 2], f32)
            nc.sync.dma_start(wsk_t[:], wsk_e[:])
            wsvc_t = constp.tile([128, 2], f32)
            nc.sync.dma_start(wsvc_t[:], wsvc_e[:])
            bq_t = constp.tile([128, 2], f32)
            nc.sync.dma_start(bq_t[:], bq_e[:])
            bk_t = constp.tile([128, 2], f32)
            nc.sync.dma_start(bk_t[:], bk_e[:])
            bvc_t = constp.tile([128, 2], f32)
            nc.sync.dma_start(bvc_t[:], bvc_e[:])
            bg_t = constp.tile([128, FFL // 128], f32)
            nc.sync.dma_start(bg_t[:], bg_e[:])
            bu_t = constp.tile([128, FFL // 128], f32)
            nc.sync.dma_start(bu_t[:], bu_e[:])
            wsg_t = constp.tile([128, FFL // 128], f32)
            nc.sync.dma_start(wsg_t[:], wsg_e[:])
            wsu_t = constp.tile([128, FFL // 128], f32)
            nc.sync.dma_start(wsu_t[:], wsu_e[:])

            # persistent QKV outputs
            kT = [kqvp.tile([128, S2], bf16, name=f"kT{m}") for m in range(2)]
            qT = [kqvp.tile([128, S], bf16, name=f"qT{m}") for m in range(2)]
            vTh = [kqvp.tile([128, S], bf16, name=f"vTh{m}") for m in range(2)]
            v_mem = kqvp.tile([128, LT, J], bf16)

            # v_mem epilogue needs column-layout stats of the mem part
            rstd_col_mem = constp.tile([128, LT], f32)
            c_col_mem = constp.tile([128, LT], f32)

            # rope tables (persistent through phase 1)
            rope_loaded = [False]

            def load_rope_tables():
                if not rope_loaded[0]:
                    nc.sync.dma_start(cos_t[:], ropec_e[:, 0:S])
                    nc.sync.dma_start(sin_t[:], ropes_e[:, 0:S])
                    rope_loaded[0] = True

            def rope_packed(slices, name):
                """Apply partial rotary to up to 8 (tile, col_lo) 16-row
                rotary groups at once, packed across all 128 partitions.
                Each slice is (tile, row_lo, col_lo); processes
                tile[row_lo:row_lo+16, col_lo:col_lo+S]. The cos/sin
                tables have the same 16-row pattern on every group, and
                positions repeat across both S-halves."""
                pk = ropep.tile([128, S], bf16, name=f"pk_{name}",
                                tag="rope_pk", bufs=1)
                sw = ropep.tile([128, S], bf16, name=f"sw_{name}",
                                tag="rope_sw", bufs=1)
                for i, (t, rlo, clo) in enumerate(slices):
                    csl = slice(clo, clo + S)
                    eng = nc.sync if i % 2 == 0 else nc.scalar
                    eng.dma_start(pk[16 * i:16 * i + 16, :],
                                  t[rlo:rlo + 16, csl])
                    eng.dma_start(sw[16 * i:16 * i + 8, :],
                                  t[rlo + 8:rlo + 16, csl])
                    eng.dma_start(sw[16 * i + 8:16 * i + 16, :],
                                  t[rlo:rlo + 8, csl])
                n = 16 * len(slices)
                nc.vector.tensor_mul(pk[:n, :], pk[:n, :], cos_t[:n, :])
                nc.vector.tensor_mul(sw[:n, :], sw[:n, :], sin_t[:n, :])
                nc.vector.tensor_add(pk[:n, :], pk[:n, :], sw[:n, :])
                for i, (t, rlo, clo) in enumerate(slices):
                    csl = slice(clo, clo + S)
                    eng = nc.sync if i % 2 == 0 else nc.scalar
                    eng.dma_start(t[rlo:rlo + 16, csl],
                                  pk[16 * i:16 * i + 16, :])

            # ---------- LN1 stats (row-major, col-tiled pairs) -------------
            def stats_nb(x_nb, nb, sqp, psp, row16):
                """Per 512-block LN sums: sum -> stp[0:1] ((0,0)),
                sumsq -> stp[32:33] ((0,32)): concurrent col-tiled pairs
                at different partitions of one PSUM bank."""
                nsl = slice(nb * NBLK, (nb + 1) * NBLK)
                stp = psp.tile([128, NBLK], f32, name="stp")
                for kc in range(KC):
                    sq_t = sqp.tile([128, NBLK], bf16, name="sq_t")
                    nc.scalar.activation(sq_t[:], x_nb[:, kc, :],
                                         AF.Square)
                    nc.tensor.matmul(
                        stp[0:1, :], ones_c[:], x_nb[:, kc, :],
                        start=(kc == 0), stop=(kc == KC - 1),
                        tile_position=(0, 0))
                    nc.tensor.matmul(
                        stp[32:33, :], ones_c[:], sq_t[:],
                        start=(kc == 0), stop=(kc == KC - 1),
                        tile_position=(0, 32))
                nc.vector.tensor_copy(row16[0:1, nsl], stp[0:1, :])
                nc.vector.tensor_copy(row16[32:33, nsl], stp[32:33, :])

            def ln_stats_fin(row16, psp, rowp, part_name, want_col):
                """Broadcast the raw sums (1/D folded into the broadcast
                constant) and finish the stats math on full [128, S]
                tiles. Returns (rstd_b, c_b)."""
                mean_b = rowp.tile([128, S], bf16, name="mean_b",
                                   tag="meanb")
                ex2_b = rowp.tile([128, S], bf16, name="ex2_b", tag="ex2b")
                for i in range(0, S, NBLK):
                    isl = slice(i, i + NBLK)
                    ps = psp.tile([128, NBLK], f32, name="bc_ps",
                                  tag="bc_ps", bufs=1)
                    nc.tensor.matmul(ps[:], invD128[0:1, :],
                                     row16[0:1, isl],
                                     start=True, stop=True)
                    nc.vector.tensor_copy(mean_b[:, isl], ps[:])
                    ps2 = psp.tile([128, NBLK], f32, name="bc_ps2",
                                   tag="bc_ps", bufs=1)
                    nc.tensor.matmul(ps2[:], invD128[32:33, :],
                                     row16[32:33, isl],
                                     start=True, stop=True)
                    nc.vector.tensor_copy(ex2_b[:, isl], ps2[:])
                m2_b = rowp.tile([128, S], bf16, name="m2_b", tag="m2b")
                nc.vector.tensor_mul(m2_b[:], mean_b[:], mean_b[:])
                nc.vector.tensor_sub(ex2_b[:], ex2_b[:], m2_b[:])
                rstd_b = rowp.tile([128, S], bf16, name="rstd_b",
                                   tag="rstdb")
                nc.scalar.activation(ex2_b[:], ex2_b[:], AF.Ln,
                                     bias=eps_c[:])
                nc.scalar.activation(rstd_b[:], ex2_b[:], AF.Exp,
                                     scale=-0.5)
                c_b = rowp.tile([128, S], bf16, name="c_b", tag="cb")
                nc.vector.tensor_mul(c_b[:], mean_b[:], rstd_b[:])
                if want_col:
                    # round-trip on the vector queue so the xh loads on
                    # sync/scalar are not stuck behind this stats chain
                    for nm, row, col in (("rstd", rstd_b, rstd_col_mem),
                                         ("c", c_b, c_col_mem)):
                        dr = dramp.tile([S], bf16,
                                        name=f"st_{nm}_{part_name}")
                        nc.gpsimd.dma_start(
                            dr[:].rearrange("(o a) -> o a", o=1),
                            row[0:1, :])
                        col16 = rowp.tile([128, LT], bf16,
                                          name=f"c16_{nm}", tag="col16")
                        nc.gpsimd.dma_start(
                            col16[:],
                            dr[:].rearrange("(t p) -> p t", p=128))
                        nc.vector.tensor_copy(col[:], col16[:])
                return rstd_b, c_b

            def fin_nb(row16, nb, scrp, psp):
                """Per-512-block stats finalize: broadcast raw sums and
                produce (rstd, c) tiles for just these columns, so the
                epilogues + rope for block nb can run while later blocks
                still project."""
                nsl = slice(nb * NBLK, (nb + 1) * NBLK)
                mean5 = scrp.tile([128, NBLK], bf16, name="mean5",
                                  tag="f_mean", bufs=2)
                ex25 = scrp.tile([128, NBLK], bf16, name="ex25",
                                 tag="f_ex2", bufs=2)
                ps = psp.tile([128, NBLK], f32, name="bc_ps",
                              tag="bc_ps", bufs=1)
                nc.tensor.matmul(ps[:], invD128[0:1, :], row16[0:1, nsl],
                                 start=True, stop=True)
                nc.vector.tensor_copy(mean5[:], ps[:])
                ps2 = psp.tile([128, NBLK], f32, name="bc_ps2",
                               tag="bc_ps", bufs=1)
                nc.tensor.matmul(ps2[:], invD128[32:33, :],
                                 row16[32:33, nsl],
                                 start=True, stop=True)
                nc.vector.tensor_copy(ex25[:], ps2[:])
                m25 = scrp.tile([128, NBLK], bf16, name="m25",
                                tag="f_m2", bufs=2)
                nc.vector.tensor_mul(m25[:], mean5[:], mean5[:])
                nc.vector.tensor_sub(ex25[:], ex25[:], m25[:])
                nc.scalar.activation(ex25[:], ex25[:], AF.Ln,
                                     bias=eps_c[:])
                rstd5 = scrp.tile([128, NBLK], bf16, name="rstd5",
                                  tag="f_rstd", bufs=2)
                nc.scalar.activation(rstd5[:], ex25[:], AF.Exp,
                                     scale=-0.5)
                c5 = scrp.tile([128, NBLK], bf16, name="c5",
                               tag="f_c", bufs=2)
                nc.vector.tensor_mul(c5[:], mean5[:], rstd5[:])
                return rstd5, c5

            def proj_epi_nb(dst, dst_off, nb, rstd5, c5, ws_t, b_t,
                            scrp):
                """LN epilogue for one 512-block: d = d*rstd-(c*ws-b)."""
                for m in range(2):
                    d = dst[m][:, dst_off + nb * NBLK:
                               dst_off + (nb + 1) * NBLK]
                    cw = scrp.tile([128, NBLK], bf16, name="cw_nb",
                                   tag="cw_nb", bufs=2)
                    nc.vector.tensor_scalar(
                        out=cw[:], in0=c5[:],
                        scalar1=ws_t[:, m:m + 1],
                        scalar2=b_t[:, m:m + 1],
                        op0=Alu.mult, op1=Alu.subtract)
                    nc.vector.tensor_mul(d, d, rstd5[:])
                    nc.vector.tensor_sub(d, d, cw[:])

            def rope_packed_nb(slices, name, nb):
                """rope_packed restricted to one 512-column block."""
                nsl = slice(nb * NBLK, (nb + 1) * NBLK)
                pk = ropep.tile([128, NBLK], bf16, name=f"pk_{name}",
                                tag="rope_pk", bufs=2)
                sw = ropep.tile([128, NBLK], bf16, name=f"sw_{name}",
                                tag="rope_sw", bufs=2)
                for i, (t, rlo, clo) in enumerate(slices):
                    csl = slice(clo + nb * NBLK, clo + (nb + 1) * NBLK)
                    eng = nc.sync if i % 2 == 0 else nc.scalar
                    eng.dma_start(pk[16 * i:16 * i + 16, :],
                                  t[rlo:rlo + 16, csl])
                    eng.dma_start(sw[16 * i:16 * i + 8, :],
                                  t[rlo + 8:rlo + 16, csl])
                    eng.dma_start(sw[16 * i + 8:16 * i + 16, :],
                                  t[rlo:rlo + 8, csl])
                n = 16 * len(slices)
                nc.vector.tensor_mul(pk[:n, :], pk[:n, :],
                                     cos_t[:n, nsl])
                nc.vector.tensor_mul(sw[:n, :], sw[:n, :],
                                     sin_t[:n, nsl])
                nc.vector.tensor_add(pk[:n, :], pk[:n, :], sw[:n, :])
                for i, (t, rlo, clo) in enumerate(slices):
                    csl = slice(clo + nb * NBLK, clo + (nb + 1) * NBLK)
                    eng = nc.sync if i % 2 == 0 else nc.scalar
                    eng.dma_start(t[rlo:rlo + 16, csl],
                                  pk[16 * i:16 * i + 16, :])

            def proj_raw_nb(wt, dst, dst_off, x_nb, nb, psp):
                """Raw projection matmuls for one 512-column block,
                copied to dst bf16 (no LN dependency)."""
                for m in range(2):
                    ps = psp.tile([128, NBLK], f32, name="proj_ps")
                    for kc in range(KC):
                        nc.tensor.matmul(
                            ps[:],
                            wt[:, kc, m * 128:(m + 1) * 128],
                            x_nb[:, kc, :],
                            start=(kc == 0), stop=(kc == KC - 1))
                    d = dst[m][:, dst_off + nb * NBLK:
                               dst_off + (nb + 1) * NBLK]
                    nc.vector.tensor_copy(d, ps[:])

            def proj_epi(dst, dst_off, rstd_b, c_b, ws_t, b_t, scrp):
                """LN epilogue in place: d = d*rstd - (c*ws - bias)."""
                for m in range(2):
                    for nb in range(NB):
                        sl = slice(nb * NBLK, (nb + 1) * NBLK)
                        d = dst[m][:, dst_off + nb * NBLK:
                                   dst_off + (nb + 1) * NBLK]
                        cw = scrp.tile([128, NBLK], bf16, name="cw_nb",
                                       tag="cw_nb", bufs=2)
                        nc.vector.tensor_scalar(
                            out=cw[:], in0=c_b[:, sl],
                            scalar1=ws_t[:, m:m + 1],
                            scalar2=b_t[:, m:m + 1],
                            op0=Alu.mult, op1=Alu.subtract)
                        nc.vector.tensor_mul(d, d, rstd_b[:, sl])
                        nc.vector.tensor_sub(d, d, cw[:])

            with (
                tc.tile_pool(name="wqkv", bufs=1) as wqkvp,
                tc.tile_pool(name="psq", bufs=2, space="PSUM") as psqp,
                tc.tile_pool(name="psst", bufs=2, space="PSUM") as psstp,
            ):
                wq_t = wqkvp.tile([128, KC, J], bf16)
                wk_t = wqkvp.tile([128, KC, J], bf16)
                wv_t = wqkvp.tile([128, KC, J], bf16)

                # ----- phase 1: both parts with per-512-block x tiles.
                # The mem-part finalize (stats math + k/v epilogues, a
                # long serial DVE chain) is emitted inside the hid loop
                # so it overlaps the hid projection streams. -----
                ropep = tc.alloc_tile_pool(name="rope", bufs=1)
                cos_t = ropep.tile([128, S], bf16)
                sin_t = ropep.tile([128, S], bf16)
                with (
                    tc.tile_pool(name="xm", bufs=2) as xmp,
                    tc.tile_pool(name="sqa", bufs=8) as sqap,
                    tc.tile_pool(name="rowa", bufs=1) as rowap,
                ):
                    nc.sync.dma_start(wk_t[:], wk_e[:])
                    nc.scalar.dma_start(wv_t[:], wv_e[:])
                    nc.sync.dma_start(wq_t[:], wq_e[:])
                    row16a = rowap.tile([128, S], bf16, name="r16_mem",
                                        tag="r16m")
                    row16b = rowap.tile([128, S], bf16, name="r16_hid",
                                        tag="r16h")
                    for nb in range(NB):
                        x_nb = xmp.tile([128, KC, NBLK], bf16,
                                        name="x_nb")
                        for kc in range(KC):
                            eng = nc.sync if kc % 2 == 0 else nc.scalar
                            eng.dma_start(
                                x_nb[:, kc, :],
                                xm_e[:, kc, nb * NBLK:(nb + 1) * NBLK])
                        proj_raw_nb(wk_t, kT, 0, x_nb, nb, psqp)
                        # v_mem row-major: lhsT = xm l-tile, rhs = wv
                        for li in range(4):
                            lt = nb * 4 + li
                            ps = psqp.tile([128, J], f32, name="vm_ps",
                                           bufs=2)
                            for kc in range(KC):
                                nc.tensor.matmul(
                                    ps[:],
                                    x_nb[:, kc, li * 128:(li + 1) * 128],
                                    wv_t[:, kc, :],
                                    start=(kc == 0), stop=(kc == KC - 1))
                            nc.vector.tensor_copy(v_mem[:, lt, :], ps[:])
                        stats_nb(x_nb, nb, sqap, psstp, row16a)
                    for nb in range(NB):
                        x_nb = xmp.tile([128, KC, NBLK], bf16,
                                        name="x_nb")
                        for kc in range(KC):
                            eng = nc.sync if kc % 2 == 0 else nc.scalar
                            eng.dma_start(
                                x_nb[:, kc, :],
                                xh_e[:, kc, nb * NBLK:(nb + 1) * NBLK])
                        if nb == 0:
                            load_rope_tables()
                        proj_raw_nb(wq_t, qT, 0, x_nb, nb, psqp)
                        proj_raw_nb(wk_t, kT, S, x_nb, nb, psqp)
                        proj_raw_nb(wv_t, vTh, 0, x_nb, nb, psqp)
                        stats_nb(x_nb, nb, sqap, psstp, row16b)
                        # per-block hid finalize: epilogues + rope for
                        # block nb run while later blocks still project
                        rstd5, c5 = fin_nb(row16b, nb, sqap, psstp)
                        proj_epi_nb(qT, 0, nb, rstd5, c5, wsq_t, bq_t,
                                    sqap)
                        rope_packed_nb(
                            [(qT[0], 0, 0), (qT[0], 64, 0),
                             (qT[1], 0, 0), (qT[1], 64, 0),
                             (kT[0], 0, 0), (kT[0], 64, 0),
                             (kT[1], 0, 0), (kT[1], 64, 0)], "a", nb)
                        proj_epi_nb(kT, S, nb, rstd5, c5, wsk_t, bk_t,
                                    sqap)
                        rope_packed_nb(
                            [(kT[0], 0, S), (kT[0], 64, S),
                             (kT[1], 0, S), (kT[1], 64, S)], "b", nb)
                        proj_epi_nb(vTh, 0, nb, rstd5, c5, wsvc_t, bvc_t,
                                    sqap)
                        if nb == 0:
                            bcast_rows(wsvb, wsv_row, J, psqp, onesf)
                            bcast_rows(bvb, bv_row, J, psqp, onesf)
                        # per-block MEM finalize: k epilogue + v_mem
                        # epilogue for mem-block nb
                        rstd5m, c5m = fin_nb(row16a, nb, sqap, psstp)
                        proj_epi_nb(kT, 0, nb, rstd5m, c5m, wsk_t, bk_t,
                                    sqap)
                        # column-layout stats for the v_mem epilogue via
                        # a tiny DRAM round-trip on the gpsimd queue
                        for nm, row, col in (
                                ("rstd", rstd5m, rstd_col_mem),
                                ("c", c5m, c_col_mem)):
                            dr = dramp.tile([NBLK], bf16,
                                            name=f"st_{nm}_{nb}")
                            nc.gpsimd.dma_start(
                                dr[:].rearrange("(o a) -> o a", o=1),
                                row[0:1, :])
                            col4 = sqap.tile([128, 4], bf16,
                                             name=f"c4_{nm}",
                                             tag="col4", bufs=4)
                            nc.gpsimd.dma_start(
                                col4[:],
                                dr[:].rearrange("(t p) -> p t", p=128))
                            nc.vector.tensor_copy(
                                col[:, 4 * nb:4 * nb + 4], col4[:])
                        for li in range(4):
                            lt = nb * 4 + li
                            cwv = sqap.tile([128, J], f32,
                                            name="cwv", bufs=2)
                            nc.vector.tensor_scalar(
                                out=cwv[:], in0=wsvb[:],
                                scalar1=c_col_mem[:, lt:lt + 1],
                                scalar2=None, op0=Alu.mult)
                            nc.vector.scalar_tensor_tensor(
                                out=v_mem[:, lt, :],
                                in0=v_mem[:, lt, :],
                                scalar=rstd_col_mem[:, lt:lt + 1],
                                in1=cwv[:], op0=Alu.mult,
                                op1=Alu.subtract)
                            nc.vector.tensor_add(
                                v_mem[:, lt, :], v_mem[:, lt, :],
                                bvb[:])
                ropep.release()

            # ---------- loop 1: attention + local o_p + AllReduce ---------
            op_dram = [dramp.tile([D, NBLK], bf16, name=f"op_d{b}")
                       for b in range(NB)]
            h_sh = [dshp.tile([D, NBLK], bf16, name=f"h_sh{b}",
                              addr_space="Shared")
                    for b in range(NB)]
            with (
                tc.tile_pool(name="maskp", bufs=1) as maskp,
                tc.tile_pool(name="wop", bufs=1) as wop,
                tc.tile_pool(name="attw", bufs=8) as attwp,
                tc.tile_pool(name="attt", bufs=4) as atttp,
                tc.tile_pool(name="cmbp", bufs=3) as cmbp,
                tc.tile_pool(name="attr", bufs=1) as attrp,
                tc.tile_pool(name="oc", bufs=2) as ocp,
                tc.tile_pool(name="psS", bufs=4, space="PSUM") as psSp,
                tc.tile_pool(name="psA", bufs=1, space="PSUM") as psAp,
                tc.tile_pool(name="psD", bufs=1, space="PSUM") as psDp,
            ):
                masks_t = maskp.tile([128, 4, NBLK], bf16)
                nc.sync.dma_start(masks_t[:], masks_e[:])
                h0sq_t = maskp.tile([128, KC, NBLK], bf16)
                wo_t = wop.tile([128, MD, D], bf16)
                nc.sync.dma_start(wo_t[:], wo_e[:])
                for b in range(NB):
                    bsl = slice(b * NBLK, (b + 1) * NBLK)
                    T = 4 * b + 4
                    if b == 1:
                        # prefetch gate/up first halves behind o stores
                        nc.sync.dma_start(wg_h1[:],
                                          wg_e[:, :, :FFL // 2])
                        nc.scalar.dma_start(wu_h1[:],
                                            wu_e[:, :, :FFL // 2])
                    if b == 3:
                        # h0 = o_sum(0) + xh(0) and its squares, built on
                        # DMA + gpsimd only (Scalar is block 3's pacer)
                        for tq in range(4):
                            eng = nc.sync if tq % 2 == 0 else nc.scalar
                            eng.dma_start(
                                h0_t[:, tq * 4:(tq + 1) * 4, :],
                                h_sh[0][tq * 512:(tq + 1) * 512, :]
                                .rearrange("(t p) s -> p t s", p=128))
                        for kc in range(KC):
                            xh0 = atttp.tile([128, NBLK], bf16,
                                             name="xh0", tag="xh0",
                                             bufs=2)
                            eng = nc.sync if kc % 2 == 0 else nc.scalar
                            eng.dma_start(xh0[:], xh_e[:, kc, 0:NBLK])
                            nc.gpsimd.tensor_add(h0_t[:, kc, :],
                                                 h0_t[:, kc, :], xh0[:])
                            nc.gpsimd.tensor_mul(h0sq_t[:, kc, :],
                                                 h0_t[:, kc, :],
                                                 h0_t[:, kc, :])
                    den4 = psDp.tile([128, NBLK], f32, name="den4")
                    sf4 = psSp.tile([128, NBLK], f32, name="sf4",
                                    tag="sbrb", bufs=1)
                    dent4 = attrp.tile([128, NBLK], f32, name="dent4")
                    swb4 = attrp.tile([128, NBLK], bf16, name="swb4")
                    rcpb4 = attrp.tile([128, NBLK], bf16, name="rcpb4")
                    ap_ps = [psAp.tile([128, NBLK], f32, name=f"ap{m}",
                                       bufs=1)
                             for m in range(2)]
                    # self-key q.k products hoisted: they only need the
                    # rope'd q/k, so the DVE does them while the PE runs
                    # the score matmuls; one full-tile mul covers both
                    # heads of an m group
                    qks = []
                    for m in range(2):
                        qk = atttp.tile([128, NBLK], bf16,
                                        name=f"qk{m}",
                                        tag=f"qk{m}", bufs=2)
                        nc.vector.tensor_mul(
                            qk[:, :], qT[m][:, bsl],
                            kT[m][:, S + b * NBLK:S + (b + 1) * NBLK])
                        qks.append(qk)

                    def q_lo(t):
                        """First unmasked q column for key-tile t (the
                        diagonal band is strictly causal: keys t*128+ii
                        only reach q > t*128+ii within the block)."""
                        return max(0, (t - 4 * b) * 128)

                    def s_pair(t):
                        """Score MMs for key-tile t, all 4 heads: two
                        row-tiled concurrent pairs, band-narrowed."""
                        tsl = slice(t * 128, (t + 1) * 128)
                        c0 = q_lo(t)
                        ss = []
                        for m in range(2):
                            for o in (0, 64):
                                hsl = slice(o, o + 64)
                                s_ps = psSp.tile([128, NBLK], f32,
                                                 name="s_ps")
                                nc.tensor.matmul(
                                    s_ps[:, c0:], kT[m][hsl, tsl],
                                    qT[m][hsl,
                                          b * NBLK + c0:(b + 1) * NBLK],
                                    start=True, stop=True,
                                    tile_position=(o, 0))
                                ss.append(s_ps)
                        return ss

                    ss_cur = s_pair(0)
                    for t in range(T):
                        ss_next = s_pair(t + 1) if t + 1 < T else None
                        c0 = q_lo(t)
                        # exp (+ mask on the diagonal band) on ScalarE/DVE
                        ws = []
                        for i, (m, o) in enumerate(
                                ((0, 0), (0, 64), (1, 0), (1, 64))):
                            w_t = attwp.tile([128, NBLK], bf16, name="w_t")
                            nc.scalar.activation(
                                w_t[:, c0:], ss_cur[i][:, c0:], AF.Exp,
                                scale=0.125)
                            if t >= 4 * b:
                                nc.vector.tensor_mul(
                                    w_t[:, c0:], w_t[:, c0:],
                                    masks_t[:, t - 4 * b, c0:])
                            ws.append(w_t)
                        # AV: col-tiled concurrent pairs per m
                        for m in range(2):
                            for io, o in enumerate((0, 64)):
                                nc.tensor.matmul(
                                    ap_ps[m][o:o + 64, c0:],
                                    v_mem[:, t,
                                          m * 128 + o:m * 128 + o + 64],
                                    ws[2 * m + io][:, c0:],
                                    start=(t == 0), stop=(t == T - 1),
                                    tile_position=(0, o))
                        # denominators: concurrent pairs at cols r
                        for m in range(2):
                            for io, o in enumerate((0, 64)):
                                r = 32 * (2 * m + io)
                                nc.tensor.matmul(
                                    den4[r:r + 1, c0:], ones_c[:, 0:1],
                                    ws[2 * m + io][:, c0:],
                                    start=(t == 0), stop=(t == T - 1),
                                    tile_position=(0, r))
                        ss_cur = ss_next

                    # self key: sf matmuls per head, then the whole
                    # denominator chain as full-tile ops (the valid rows
                    # sit at partitions 0/32/64/96; the other partitions
                    # carry garbage that is never read back)
                    heads = [(m, io, o) for m in range(2)
                             for io, o in enumerate((0, 64))]
                    for m, io, o in heads:
                        hsl = slice(o, o + 64)
                        nc.tensor.matmul(
                            sf4[32 * (2 * m + io):32 * (2 * m + io) + 1,
                                :],
                            ones_c[hsl, 0:1], qks[m][hsl, :],
                            start=True, stop=True,
                            tile_position=(o, 32 * (2 * m + io)))
                    nc.scalar.activation(swb4[:], sf4[:], AF.Exp,
                                         scale=0.125)
                    nc.vector.tensor_add(dent4[:], den4[:], swb4[:])
                    lnd = attrp.tile([128, NBLK], f32, name="lnd")
                    nc.scalar.activation(lnd[:], dent4[:], AF.Ln)
                    nc.scalar.activation(rcpb4[:], lnd[:], AF.Exp,
                                         scale=-1.0)
                    # broadcast self_w and 1/den to each head's 64 rows
                    for m in range(2):
                        sb_ps = psSp.tile([128, NBLK], f32, name="sb_ps",
                                          tag="sbrb", bufs=1)
                        rb_ps = psSp.tile([128, NBLK], f32, name="rb_ps",
                                          tag="sbrb", bufs=1)
                        for io, o in enumerate((0, 64)):
                            r = 32 * (2 * m + io)
                            rsl = slice(r, r + 1)
                            nc.tensor.matmul(
                                sb_ps[o:o + 64, :], ones128[rsl, 0:64],
                                swb4[rsl, :], start=True, stop=True,
                                tile_position=(r, o))
                            nc.tensor.matmul(
                                rb_ps[o:o + 64, :], ones128[rsl, 0:64],
                                rcpb4[rsl, :], start=True, stop=True,
                                tile_position=(r, o))
                        # combine: (attn + self_w * vTh) / den
                        t0 = atttp.tile([128, NBLK], bf16, name="t0")
                        nc.vector.tensor_mul(t0[:], vTh[m][:, bsl],
                                             sb_ps[:])
                        t1 = atttp.tile([128, NBLK], bf16, name="t1")
                        nc.vector.tensor_add(t1[:], ap_ps[m][:], t0[:])
                        cmb = cmbp.tile([128, NBLK], bf16, name=f"cmb{m}")
                        nc.vector.tensor_mul(cmb[:], t1[:], rb_ps[:])
                        if m == 0:
                            cmb0 = cmb
                        else:
                            cmb1 = cmb

                    # local o_p: full-D partial from this core's heads
                    # (attention only; the residual joins in the down-proj
                    # partial instead), stored in 512-row quarters
                    for q in range(4):
                        oc_q = ocp.tile([128, 4, NBLK], bf16,
                                        name="oc_q", tag="oc_q", bufs=2)
                        for sub in range(4):
                            md16 = q * 4 + sub
                            ps = psSp.tile([128, NBLK], f32, name="o_ps",
                                           tag="s_ps")
                            nc.tensor.matmul(
                                ps[:],
                                wo_t[:, 0, md16 * 128:(md16 + 1) * 128],
                                cmb0[:], start=True, stop=False)
                            nc.tensor.matmul(
                                ps[:],
                                wo_t[:, 1, md16 * 128:(md16 + 1) * 128],
                                cmb1[:], start=False, stop=True)
                            if sub % 2 == 0:
                                nc.vector.tensor_copy(
                                    oc_q[:, sub, :], ps[:])
                            else:
                                nc.scalar.copy(oc_q[:, sub, :], ps[:])
                        eng = nc.sync if q % 2 == 0 else nc.scalar
                        eng.dma_start(
                            op_dram[b][q * 512:(q + 1) * 512, :].rearrange(
                                "(t p) s -> p t s", p=128),
                            oc_q[:])
                    # one AllReduce per block: every core gets the full
                    # o-sum for this block's columns
                    nc.gpsimd.collective_compute(
                        "AllReduce", mybir.AluOpType.add,
                        replica_groups=rg,
                        ins=[op_dram[b].opt()], outs=[h_sh[b].opt()])
                    if b == 3:
                        # LN2 stats for block 0 (PE/DVE/2 Scalar ops run
                        # in block 3's engine slack / the AR3 window)
                        st0 = psSp.tile([128, NBLK], f32, name="st0",
                                        tag="sbrb", bufs=1)
                        for kc in range(KC):
                            nc.tensor.matmul(
                                st0[0:1, :], ones_c[:], h0_t[:, kc, :],
                                start=(kc == 0), stop=(kc == KC - 1),
                                tile_position=(0, 0))
                            nc.tensor.matmul(
                                st0[32:33, :], ones_c[:],
                                h0sq_t[:, kc, :],
                                start=(kc == 0), stop=(kc == KC - 1),
                                tile_position=(0, 32))
                        r160 = atttp.tile([128, NBLK], bf16,
                                          name="r160", tag="r160",
                                          bufs=1)
                        nc.vector.tensor_copy(r160[0:1, :], st0[0:1, :])
                        nc.vector.tensor_copy(r160[32:33, :],
                                              st0[32:33, :])
                        ps0 = psSp.tile([128, NBLK], f32, name="bc0",
                                        tag="sbrb", bufs=1)
                        nc.tensor.matmul(ps0[:], invD128[0:1, :],
                                         r160[0:1, :],
                                         start=True, stop=True)
                        mean0 = atttp.tile([128, NBLK], bf16,
                                           name="mean0", tag="mean0",
                                           bufs=1)
                        nc.vector.tensor_copy(mean0[:], ps0[:])
                        ps0b = psSp.tile([128, NBLK], f32, name="bc0b",
                                         tag="sbrb", bufs=1)
                        nc.tensor.matmul(ps0b[:], invD128[32:33, :],
                                         r160[32:33, :],
                                         start=True, stop=True)
                        ex20 = atttp.tile([128, NBLK], bf16,
                                          name="ex20", tag="ex20",
                                          bufs=1)
                        nc.vector.tensor_copy(ex20[:], ps0b[:])
                        m20 = atttp.tile([128, NBLK], bf16, name="m20",
                                         tag="m20", bufs=1)
                        nc.vector.tensor_mul(m20[:], mean0[:], mean0[:])
                        nc.vector.tensor_sub(ex20[:], ex20[:], m20[:])
                        nc.scalar.activation(ex20[:], ex20[:], AF.Ln,
                                             bias=eps_c[:])
                        nc.scalar.activation(rstd05[:], ex20[:], AF.Exp,
                                             scale=-0.5)
                        nc.vector.tensor_mul(c05[:], mean0[:],
                                             rstd05[:])
            kqvp.release()

            # second halves of gate/up + the down weight load into the
            # space the attention pools and kqv freed
            wudp = tc.alloc_tile_pool(name="wud", bufs=1)
            wg_h2 = wudp.tile([128, KC, FFL // 2], bf16)
            wu_h2 = wudp.tile([128, KC, FFL // 2], bf16)
            wd_t = wudp.tile([128, FFL // 128, D], bf16)

            # ---------- loop 2: LN2 + gated MLP + down + out --------------
            # row-parallel down: each core contracts its own FFL slice of
            # m into a full-D partial which also carries ident8 @ h (the
            # residual + o_sum, scaled 1/8); the per-block ReduceScatter
            # then hands back this core's own rows of h + down_sum — the
            # final output rows, with no separate residual path.
            dp_dram = [dramp.tile([D, NBLK], bf16, name=f"dp_d{b}")
                       for b in range(NB - 1)]
            dp_rs = [dramp.tile([J, NBLK], bf16, name=f"dp_rs{b}")
                     for b in range(NB - 1)]
            dp3_dram = [dramp.tile([D, NBLK // 4], bf16, name=f"dp3_d{i}")
                        for i in range(4)]
            dp3_rs = [dramp.tile([J, NBLK // 4], bf16, name=f"dp3_rs{i}")
                      for i in range(4)]
            with (
                tc.tile_pool(name="hblk", bufs=2) as hblkp,
                tc.tile_pool(name="xh2", bufs=2) as xh2p,
                tc.tile_pool(name="sq5", bufs=2) as sq5p,
                tc.tile_pool(name="sm5", bufs=1) as sm5p,
                tc.tile_pool(name="mloc", bufs=1) as mlocp,
                tc.tile_pool(name="gut", bufs=2) as gutp,
                tc.tile_pool(name="dcp", bufs=2) as dcp,
                tc.tile_pool(name="outt", bufs=1) as outtp,
                tc.tile_pool(name="psG", bufs=2, space="PSUM") as psGp,
                tc.tile_pool(name="psU", bufs=2, space="PSUM") as psUp,
                tc.tile_pool(name="psst5", bufs=1, space="PSUM") as psst5p,
                tc.tile_pool(name="psDn", bufs=2, space="PSUM") as psDnp,
            ):
                def h_load(b):
                    """h_t = o_sum (AllReduced) + xh, raw pre-LN2."""
                    bsl = slice(b * NBLK, (b + 1) * NBLK)
                    h_t = hblkp.tile([128, KC, NBLK], bf16, name="h_t")
                    for tq in range(4):
                        eng = nc.sync if tq % 2 == 0 else nc.scalar
                        eng.dma_start(
                            h_t[:, tq * 4:(tq + 1) * 4, :],
                            h_sh[b][tq * 512:(tq + 1) * 512, :].rearrange(
                                "(t p) s -> p t s", p=128))
                    for kc in range(KC):
                        xh2 = xh2p.tile([128, NBLK], bf16, name="xh2")
                        eng = nc.sync if kc % 2 == 0 else nc.scalar
                        eng.dma_start(xh2[:], xh_e[:, kc, bsl])
                        # adds on the (otherwise idle) gpsimd engine so
                        # the h chain never queues behind DVE work
                        nc.gpsimd.tensor_add(h_t[:, kc, :],
                                             h_t[:, kc, :], xh2[:])
                    return h_t

                def down_block(h_t, m_loc, clo, chi, dpd, dpr):
                    """Down partial over columns [clo, chi) + ident8 @ h;
                    ReduceScatter returns own rows of h + down_sum."""
                    w = chi - clo
                    for q in range(4):
                        dcq = dcp.tile([128, 4, NBLK], bf16, name="dcq")
                        for sub in range(4):
                            md16 = q * 4 + sub
                            ps = psDnp.tile([128, NBLK], f32, name="d_ps")
                            nc.tensor.matmul(
                                ps[:, :w], ident8[:],
                                h_t[:, md16, clo:chi],
                                start=True, stop=False)
                            for fc in range(FFL // 128):
                                nc.tensor.matmul(
                                    ps[:, :w],
                                    wd_t[:, fc,
                                         md16 * 128:(md16 + 1) * 128],
                                    m_loc[:, fc, clo:chi],
                                    start=False,
                                    stop=(fc == FFL // 128 - 1))
                            nc.scalar.copy(dcq[:, sub, :w], ps[:, :w])
                        eng = nc.sync if q % 2 == 0 else nc.scalar
                        eng.dma_start(
                            dpd[q * 512:(q + 1) * 512, :].rearrange(
                                "(t p) s -> p t s", p=128),
                            dcq[:, :, :w])
                    nc.gpsimd.collective_compute(
                        "ReduceScatter", mybir.AluOpType.add,
                        replica_groups=rg,
                        ins=[dpd.opt()], outs=[dpr.opt()])

                def ln2_block(b, h_t):
                    """LN2 stats on raw h; returns broadcast (rstd, c)
                    tiles. gu runs on RAW h with the LN correction folded
                    into its epilogue, so the normalize never sits on the
                    inter-block critical path."""
                    stp = psst5p.tile([128, NBLK], f32, name="st5")
                    for kc in range(KC):
                        sq_t = sq5p.tile([128, NBLK], bf16, name="sq5_t",
                                          bufs=8)
                        nc.scalar.activation(sq_t[:], h_t[:, kc, :],
                                             AF.Square)
                        nc.tensor.matmul(
                            stp[0:1, :], ones_c[:], h_t[:, kc, :],
                            start=(kc == 0), stop=(kc == KC - 1),
                            tile_position=(0, 0))
                        nc.tensor.matmul(
                            stp[32:33, :], ones_c[:], sq_t[:],
                            start=(kc == 0), stop=(kc == KC - 1),
                            tile_position=(0, 32))
                    r16 = sm5p.tile([128, NBLK], bf16, name="r165",
                                    tag="sm5r", bufs=1)
                    nc.vector.tensor_copy(r16[0:1, :], stp[0:1, :])
                    nc.vector.tensor_copy(r16[32:33, :], stp[32:33, :])
                    mean_b = sm5p.tile([128, NBLK], bf16, name="mean5b",
                                       tag="sm5m", bufs=1)
                    ex2_b = sm5p.tile([128, NBLK], bf16, name="ex25b",
                                      tag="sm5e", bufs=1)
                    ps = psst5p.tile([128, NBLK], f32, name="bc_ps",
                                     tag="bc_ps", bufs=1)
                    nc.tensor.matmul(ps[:], invD128[0:1, :], r16[0:1, :],
                                     start=True, stop=True)
                    nc.vector.tensor_copy(mean_b[:], ps[:])
                    ps2 = psst5p.tile([128, NBLK], f32, name="bc_ps2",
                                      tag="bc_ps", bufs=1)
                    nc.tensor.matmul(ps2[:], invD128[32:33, :],
                                     r16[32:33, :],
                                     start=True, stop=True)
                    nc.vector.tensor_copy(ex2_b[:], ps2[:])
                    m2_b = sm5p.tile([128, NBLK], bf16, name="m25b",
                                     tag="sm5m2", bufs=1)
                    nc.vector.tensor_mul(m2_b[:], mean_b[:], mean_b[:])
                    nc.vector.tensor_sub(ex2_b[:], ex2_b[:], m2_b[:])
                    rstd_b = sm5p.tile([128, NBLK], bf16, name="rstd5b",
                                       tag="sm5rs", bufs=2)
                    lnv5 = sm5p.tile([128, NBLK], f32, name="lnv5",
                                     tag="sm5ln", bufs=1)
                    nc.scalar.activation(lnv5[:], ex2_b[:], AF.Ln,
                                         bias=eps_c[:])
                    nc.scalar.activation(rstd_b[:], lnv5[:], AF.Exp,
                                         scale=-0.5)
                    c_bb = sm5p.tile([128, NBLK], bf16, name="c5b",
                                     tag="sm5c", bufs=2)
                    nc.vector.tensor_mul(c_bb[:], mean_b[:], rstd_b[:])
                    return rstd_b, c_bb

                def gu_block(b, h_t, rstd_b, c_bb):
                    """Gate/up on RAW h; LN2 folded into the epilogue:
                    g' = psg*rstd - (c*wsg - bg), same for u."""
                    m_loc = mlocp.tile([128, FFL // 128, NBLK], bf16,
                                       name="m_loc")
                    for mf in range(FFL // 128):
                        if mf < FFL // 256:
                            wgs, wus, mfl = wg_h1, wu_h1, mf
                        else:
                            wgs, wus, mfl = wg_h2, wu_h2, mf - FFL // 256
                        psg = psGp.tile([128, NBLK], f32, name="g_ps")
                        psu = psUp.tile([128, NBLK], f32, name="u_ps")
                        for kc in range(KC):
                            nc.tensor.matmul(
                                psg[:],
                                wgs[:, kc, mfl * 128:(mfl + 1) * 128],
                                h_t[:, kc, :],
                                start=(kc == 0), stop=(kc == KC - 1))
                            nc.tensor.matmul(
                                psu[:],
                                wus[:, kc, mfl * 128:(mfl + 1) * 128],
                                h_t[:, kc, :],
                                start=(kc == 0), stop=(kc == KC - 1))
                        cwg = gutp.tile([128, NBLK], bf16, name="cwg")
                        nc.vector.tensor_scalar(
                            out=cwg[:], in0=c_bb[:],
                            scalar1=wsg_t[:, mf:mf + 1],
                            scalar2=bg_t[:, mf:mf + 1],
                            op0=Alu.mult, op1=Alu.subtract)
                        g_t = gutp.tile([128, NBLK], bf16, name="g_t")
                        nc.vector.tensor_mul(g_t[:], psg[:], rstd_b[:])
                        nc.vector.tensor_sub(g_t[:], g_t[:], cwg[:])
                        cwu = gutp.tile([128, NBLK], bf16, name="cwu")
                        nc.vector.tensor_scalar(
                            out=cwu[:], in0=c_bb[:],
                            scalar1=wsu_t[:, mf:mf + 1],
                            scalar2=bu_t[:, mf:mf + 1],
                            op0=Alu.mult, op1=Alu.subtract)
                        u_t = gutp.tile([128, NBLK], bf16, name="u_t")
                        nc.vector.tensor_mul(u_t[:], psu[:], rstd_b[:])
                        nc.vector.tensor_sub(u_t[:], u_t[:], cwu[:])
                        sg = gutp.tile([128, NBLK], bf16, name="sg")
                        nc.scalar.activation(sg[:], g_t[:], AF.Sigmoid)
                        silu = gutp.tile([128, NBLK], bf16, name="silu")
                        nc.vector.tensor_mul(silu[:], g_t[:], sg[:])
                        nc.vector.tensor_mul(m_loc[:, mf, :], silu[:],
                                             u_t[:])
                    return m_loc

                def out_block(b, clo, chi, dpr):
                    """Own jsl rows of h + down_sum -> out (f32).

                    The rsd load waits on the ReduceScatter, so it
                    rides the gpsimd queue: on sync/scalar it would
                    head-of-line block the next block's h loads and
                    Square stream behind that wait."""
                    w = chi - clo
                    rsd = outtp.tile([128, MD, NBLK], bf16, name="rsd")
                    nc.gpsimd.dma_start(
                        rsd[:, :, :w], dpr[:, :].rearrange(
                            "(t p) s -> p t s", p=128))
                    for md in range(MD):
                        ot = outtp.tile([128, NBLK], f32, name="ot")
                        nc.vector.tensor_copy(ot[:, :w], rsd[:, md, :w])
                        nc.gpsimd.dma_start(
                            out_e[md, :, b * NBLK + clo:b * NBLK + chi],
                            ot[:, :w])

                # per-block pipeline; the final block's down/RS/out is
                # split into column halves to shorten the exposed tail
                h_cur = h0_t
                st_cur = (rstd05, c05)
                nc.sync.dma_start(wg_h2[:], wg_e[:, :, FFL // 2:])
                nc.scalar.dma_start(wu_h2[:], wu_e[:, :, FFL // 2:])
                nc.sync.dma_start(wd_t[:, :FFL // 256, :],
                                  wd_e[:, :FFL // 256, :])
                nc.scalar.dma_start(wd_t[:, FFL // 256:, :],
                                    wd_e[:, FFL // 256:, :])
                # ln2(b+1)'s serial Square/stats chain is emitted before
                # gu(b)/down(b) so it overlaps their PE streams
                for b in range(NB):
                    if b + 1 < NB:
                        h_next = h_load(b + 1)
                        st_next = ln2_block(b + 1, h_next)
                    else:
                        h_next = st_next = None
                    ml = gu_block(b, h_cur, *st_cur)
                    if b < NB - 1:
                        down_block(h_cur, ml, 0, NBLK,
                                   dp_dram[b], dp_rs[b])
                        out_block(b, 0, NBLK, dp_rs[b])
                    else:
                        for qq in range(4):
                            qlo = qq * (NBLK // 4)
                            qhi = (qq + 1) * (NBLK // 4)
                            down_block(h_cur, ml, qlo, qhi,
                                       dp3_dram[qq], dp3_rs[qq])
                            out_block(b, qlo, qhi, dp3_rs[qq])
                    h_cur, st_cur = h_next, st_next
            wudp.release()
            h0p.release()
            wguh1p.release()

    return nc


# ---------------------------------------------------------------------------
# Host side
# ---------------------------------------------------------------------------

def _chunkT(a):
    """[R, D] -> [128, D//128, R] view for lhsT/rhs chunk layout.

    Result[p, kc, r] = a[r, kc*128 + p].
    """
    R, Dd = a.shape
    return np.ascontiguousarray(
        a.reshape(R, Dd // 128, 128).transpose(2, 1, 0))


def prepare_inputs(hidden_states, memory, position_ids,
                   ln1_w, ln1_b, ln2_w, ln2_b,
                   Wq, Wk, Wv, Wo, Wg, Wu, Wd, S):
    """Build the 8 per-core in_maps (numpy host prep)."""
    f32 = np.float32
    hid = np.asarray(hidden_states, f32)[0]       # [S, D]
    mem = np.asarray(memory, f32)[0]
    pos = np.asarray(position_ids)[0].astype(np.float64)

    Wq1 = np.asarray(Wq, f32) * np.asarray(ln1_w, f32)[None, :]
    Wk1 = np.asarray(Wk, f32) * np.asarray(ln1_w, f32)[None, :]
    Wv1 = np.asarray(Wv, f32) * np.asarray(ln1_w, f32)[None, :]
    bq = np.asarray(Wq, f32) @ np.asarray(ln1_b, f32)
    bk = np.asarray(Wk, f32) @ np.asarray(ln1_b, f32)
    bv = np.asarray(Wv, f32) @ np.asarray(ln1_b, f32)
    Wg2 = np.asarray(Wg, f32) * np.asarray(ln2_w, f32)[None, :]
    Wu2 = np.asarray(Wu, f32) * np.asarray(ln2_w, f32)[None, :]
    bg = np.asarray(Wg, f32) @ np.asarray(ln2_b, f32)
    bu = np.asarray(Wu, f32) @ np.asarray(ln2_b, f32)
    Wo_ = np.asarray(Wo, f32)
    Wd_ = np.asarray(Wd, f32)

    # x^T chunk layouts (shared by all cores)
    xm = _chunkT(mem).astype(BF16)                # [128, KC, S]
    xh = _chunkT(hid).astype(BF16)

    # rope tables [128, 2S], row pattern period 16
    inv = BASE ** (-(np.arange(8, dtype=np.float64) * 2) / RD)
    t = pos[:, None] * inv[None, :]               # [S, 8]
    cos8 = np.cos(t).T                            # [8, S]
    sin8 = np.sin(t).T
    cos16 = np.concatenate([cos8, cos8], 0)       # [16, S]
    sin16 = np.concatenate([-sin8, sin8], 0)
    cosf = np.tile(np.concatenate([cos16, cos16], 1), (8, 1))  # [128, 2S]
    sinf = np.tile(np.concatenate([sin16, sin16], 1), (8, 1))
    cosf = cosf.astype(BF16)
    sinf = sinf.astype(BF16)

    ident8 = (np.eye(128) * 0.125).astype(BF16)

    # strict-causal masks for the 4 diagonal-band offsets
    ii = np.arange(128)[:, None]
    jj = np.arange(NBLK)[None, :]
    masks = np.stack(
        [(ii + 128 * o < jj) for o in range(4)], 1).astype(BF16)  # [128,4,512]

    in_maps = []
    for c in range(N_CORES):
        jsl = slice(c * J, (c + 1) * J)
        fsl = slice(c * FFL, (c + 1) * FFL)
        wq_c = Wq1[jsl]                            # [J, D]
        wk_c = Wk1[jsl]
        wv_c = Wv1[jsl]
        im = {
            "xm": xm, "xh": xh,
            "ident8": ident8,
            "wq": _chunkT(wq_c).astype(BF16),
            "wk": _chunkT(wk_c).astype(BF16),
            "wv": _chunkT(wv_c).astype(BF16),
            "wo_p": _chunkT(Wo_[:, jsl]).astype(BF16),
            "wg": _chunkT(Wg2[fsl]).astype(BF16),
            "wu": _chunkT(Wu2[fsl]).astype(BF16),
            "wd": _chunkT(Wd_[:, fsl]).astype(BF16),
            "wsq": np.ascontiguousarray(
                wq_c.sum(1).reshape(MD, 128).T).astype(f32),
            "wsk": np.ascontiguousarray(
                wk_c.sum(1).reshape(MD, 128).T).astype(f32),
            "wsvc": np.ascontiguousarray(
                wv_c.sum(1).reshape(MD, 128).T).astype(f32),
            "bq": np.ascontiguousarray(
                bq[jsl].reshape(MD, 128).T).astype(f32),
            "bk": np.ascontiguousarray(
                bk[jsl].reshape(MD, 128).T).astype(f32),
            "bvc": np.ascontiguousarray(
                bv[jsl].reshape(MD, 128).T).astype(f32),
            "wsv_row": wv_c.sum(1)[None, :].astype(f32),
            "bv_row": bv[jsl][None, :].astype(f32),
            "bg": np.ascontiguousarray(
                bg[fsl].reshape(FFL // 128, 128).T).astype(f32),
            "bu": np.ascontiguousarray(
                bu[fsl].reshape(FFL // 128, 128).T).astype(f32),
            "wsg": np.ascontiguousarray(
                Wg2[fsl].sum(1).reshape(FFL // 128, 128).T).astype(f32),
            "wsu": np.ascontiguousarray(
                Wu2[fsl].sum(1).reshape(FFL // 128, 128).T).astype(f32),
            "rope_cos": cosf, "rope_sinsg": sinf,
            "masks": masks,
        }
        in_maps.append(im)
    return in_maps


def assemble_output(results, S):
    outT = np.concatenate(
        [np.asarray(results[c]["out"]).reshape(J, S)
         for c in range(N_CORES)], 0)              # [D, S]
    return np.ascontiguousarray(outT.T).reshape(1, S, D).astype(np.float32)


_GRAPH_CACHE = {}


def get_graph(S):
    if S not in _GRAPH_CACHE:
        _GRAPH_CACHE[S] = build_graph(S)
    return _GRAPH_CACHE[S]


def kernel(hidden_states, memory, attention_mask, position_ids,
           ln1_w, ln1_b, ln2_w, ln2_b, Wq, Wk, Wv, Wo, Wg, Wu, Wd):
    from concourse.bass_utils import run_bass_kernel_spmd

    S = np.asarray(hidden_states).shape[1]
    in_maps = prepare_inputs(
        hidden_states, memory, position_ids, ln1_w, ln1_b, ln2_w, ln2_b,
        Wq, Wk, Wv, Wo, Wg, Wu, Wd, S)
    nc = get_graph(S)
    res = run_bass_kernel_spmd(nc, in_maps, core_ids=list(range(N_CORES)))
    return assemble_output(res.results, S)

